# revision 1
# baseline (speedup 1.0000x reference)
"""Trainium2 Bass kernel for single-head self-attention (B=2, S=4096, D=1024).

reference:
    q = x @ Wq; k = x @ Wk; v = x @ Wv          # [B,S,D]
    energy = einsum('bid,bjd->bij', q, k) * 8.0  # SCALE = sqrt(64)
    attn = softmax(energy, axis=-1)
    out = einsum('bij,bjd->bid', attn, v) @ Wo

Two SPMD launches over 8 cores (= 2 batches x 4 query-blocks of 1024):
  phase 1: each core computes the Q/K/V projections for its own 1024
           rows only (1/8 of the total work, no redundancy); the host
           gathers K/V shards into full per-batch tensors.
  phase 2: each core runs attention + output projection for its block
           against the full K/V of its batch.

Precision: logits have std ~256 (SCALE multiplies), so softmax is
nearly an argmax -- the x->Q, x->K, Q@K^T path needs ~fp32 accuracy.
fp32 PE matmuls run at 4 cycles/row; instead those matmuls use a
hi/lo bf16 split (a = ah + al, a@b ~ ah@bh + ah@bl + al@bh) -- 3
bf16 passes at 1 cycle/row with fp32 PSUM accumulation, giving
~2^-17 relative input rounding (logit error ~0.006, negligible at
this softmax sharpness). The V path (V, P@V, @Wo) runs in plain bf16.

Layout: feature-major ("transposed") activations throughout; the host
pre-transposes x and post-transposes the output. DMA instruction count
is kept low (batched loads/stores) -- each HWDGE descriptor-generation
costs ~0.6us of serialized queue-prep time.
"""

import numpy as np
import ml_dtypes

B, S, D = 2, 4096, 1024
BLK = 1024          # queries per core
SCALE = 8.0         # HEAD_DIM ** 0.5 = sqrt(64)
NK = D // 128       # 8 k-tiles over the feature dim
NT = S // 128       # 32 j-tiles over keys
NI = BLK // 128     # 8 i-tiles over this core's queries
NJB = S // 512      # 8 key blocks of 512
BF16 = ml_dtypes.bfloat16

# phase-2 tuning knobs (swept via TimelineSim)
EPS_BUFS = 3        # PSUM banks for E accumulation
TP_BUFS = 2         # PSUM banks for PE transposes
VG = 1              # V j-tiles per DMA
VV_BUFS = 6

_cache = {}


def _split_hilo(a):
    """Split fp32 array into (hi, lo) bf16 so hi + lo ~ a with ~2^-17 rel err."""
    hi = a.astype(BF16)
    lo = (a - hi.astype(np.float32)).astype(BF16)
    return hi, lo


def _build_phase1():
    """Q/K/V projections for this core's 1024 rows."""
    import concourse.mybir as mybir
    from concourse import bacc
    from concourse.tile import TileContext

    FP32 = mybir.dt.float32
    DBF = mybir.dt.bfloat16
    SUB = mybir.AluOpType.subtract

    nc = bacc.Bacc("TRN2", target_bir_lowering=False, debug=False, num_devices=8)

    xh = nc.dram_tensor("xh", [D, BLK], DBF, kind="ExternalInput")   # rows.T hi
    xl = nc.dram_tensor("xl", [D, BLK], DBF, kind="ExternalInput")   # rows.T lo
    xb = nc.dram_tensor("xb", [NI, 128, NK, 128], DBF, kind="ExternalInput")
    wqh = nc.dram_tensor("wqh", [D, D], DBF, kind="ExternalInput")
    wql = nc.dram_tensor("wql", [D, D], DBF, kind="ExternalInput")
    wkh = nc.dram_tensor("wkh", [D, D], DBF, kind="ExternalInput")
    wkl = nc.dram_tensor("wkl", [D, D], DBF, kind="ExternalInput")
    wv = nc.dram_tensor("wv", [D, D], DBF, kind="ExternalInput")
    qth = nc.dram_tensor("qth", [D, BLK], DBF, kind="ExternalOutput")
    qtl = nc.dram_tensor("qtl", [D, BLK], DBF, kind="ExternalOutput")
    kth = nc.dram_tensor("kth", [D, BLK], DBF, kind="ExternalOutput")
    ktl = nc.dram_tensor("ktl", [D, BLK], DBF, kind="ExternalOutput")
    vo = nc.dram_tensor("vo", [NI, 128, D], DBF, kind="ExternalOutput")

    def split_mms(ps, lh, ll, rh, rl, first, last):
        # rl (the lo half of the moving operand) is consumed last so its
        # DMA can land while the two rh terms compute
        nc.tensor.matmul(ps, lhsT=lh, rhs=rh, start=first, stop=False)
        nc.tensor.matmul(ps, lhsT=ll, rhs=rh, start=False, stop=False)
        nc.tensor.matmul(ps, lhsT=lh, rhs=rl, start=False, stop=last)

    def load_w(pool, dram, name):
        # whole [D, D] weight in one DMA: [128, NK, D], slice [:, k, :]
        t = pool.tile([128, NK, D], DBF, name=name, tag=name)
        nc.sync.dma_start(t, dram[:, :].rearrange("(n p) d -> p n d", p=128))
        return t

    with TileContext(nc) as tc:
      with (
          tc.tile_pool(name="xqk", bufs=1) as xqkp,
          tc.tile_pool(name="wkqp", bufs=1) as wkqp,
          tc.tile_pool(name="wvp", bufs=1) as wvp,
          tc.tile_pool(name="xvp", bufs=1) as xvp,
          tc.tile_pool(name="vps", bufs=2, space="PSUM") as vps,
          tc.tile_pool(name="vsb", bufs=3) as vsbp,
      ):
        if True:
            # load order = consumption order; the three K-section inputs
            # are halved so the first banks' first sweep can start ~8us in
            xh_r = xh[:, :].rearrange("(n p) s -> p n s", p=128)
            xsh = xqkp.tile([128, NK, BLK], DBF, name="xsh", tag="xsh")
            w_sb = {}
            wkh_sb = wkqp.tile([128, NK, D], DBF, name="kwht", tag="kwht")
            wkh_r = wkh[:, :].rearrange("(n p) d -> p n d", p=128)
            wkl_sb = wkqp.tile([128, NK, D], DBF, name="kwlt", tag="kwlt")
            wkl_r = wkl[:, :].rearrange("(n p) d -> p n d", p=128)
            nc.sync.dma_start(xsh[:, :, 0:512], xh_r[:, :, 0:512])
            nc.sync.dma_start(wkh_sb[:, :, 0:256], wkh_r[:, :, 0:256])
            nc.sync.dma_start(wkl_sb[:, :, 0:256], wkl_r[:, :, 0:256])
            nc.sync.dma_start(wkh_sb[:, :, 256:512], wkh_r[:, :, 256:512])
            nc.sync.dma_start(wkl_sb[:, :, 256:512], wkl_r[:, :, 256:512])
            nc.sync.dma_start(xsh[:, :, 512:BLK], xh_r[:, :, 512:BLK])
            nc.sync.dma_start(wkh_sb[:, :, 512:D], wkh_r[:, :, 512:D])
            nc.sync.dma_start(wkl_sb[:, :, 512:D], wkl_r[:, :, 512:D])
            w_sb["kwht"] = wkh_sb
            w_sb["kwlt"] = wkl_sb
            xsl = xqkp.tile([128, NK, BLK], DBF, name="xsl", tag="xsl")
            nc.sync.dma_start(xsl, xl[:, :].rearrange("(n p) s -> p n s", p=128))
            wv_sb = wvp.tile([128, NK, D], DBF, name="wvt", tag="wvt")
            wv_r = wv[:, :].rearrange("(n p) d -> p n d", p=128)
            nc.sync.dma_start(wv_sb[:, :, 0:512], wv_r[:, :, 0:512])
            nc.sync.dma_start(wv_sb[:, :, 512:D], wv_r[:, :, 512:D])
            xv = xvp.tile([128, NI, NK, 128], DBF, name="xv", tag="xv")
            xb_r = xb[:, :, :, :].rearrange("j p n f -> p j n f")
            nc.sync.dma_start(xv[:, 0:2], xb_r[:, 0:2])
            nc.sync.dma_start(xv[:, 2:NI], xb_r[:, 2:NI])
            for wdram, wname in ((wqh, "qwht"), (wql, "qwlt")):
                w_sb[wname] = load_w(wkqp, wdram, wname)
        # ------------- Kt, then V, then Qt -------------
        if True:
            def v_section():
                for j in range(NI):
                    vt = vsbp.tile([128, D], DBF, name=f"vt{j}", tag="vt")
                    for db in range(2):
                        ps = vps.tile([128, 512], FP32, name=f"vps{j}_{db}", tag="vps")
                        for k in range(NK):
                            nc.tensor.matmul(
                                ps, lhsT=xv[:, j, k, :],
                                rhs=wv_sb[:, k, db * 512:(db + 1) * 512],
                                start=(k == 0), stop=(k == NK - 1),
                            )
                        nc.vector.tensor_copy(vt[:, db * 512:(db + 1) * 512], ps)
                    nc.sync.dma_start(vo[j], vt)

            for (outh, outl, pfx) in (
                (kth, ktl, "k"),
                (qth, qtl, "q"),
            ):
                with (
                    tc.tile_pool(name=f"{pfx}ps", bufs=4, space="PSUM") as psp,
                    tc.tile_pool(name=f"{pfx}st", bufs=1) as stp,
                ):
                    wh_sb = w_sb[f"{pfx}wht"]
                    wl_sb = w_sb[f"{pfx}wlt"]
                    sh_t, sl_t = [], []
                    for m in range(NK):
                        sh_t.append(stp.tile([128, BLK], DBF, name=f"{pfx}sh{m}", tag=f"sh{m}"))
                        sl_t.append(stp.tile([128, BLK], DBF, name=f"{pfx}sl{m}", tag=f"sl{m}"))
                    for nb in range(2):
                        nsl = slice(nb * 512, (nb + 1) * 512)
                        for m in range(NK):
                            msl = slice(m * 128, (m + 1) * 128)
                            ps = psp.tile([128, 512], FP32, name=f"{pfx}ps{nb}_{m}", tag="ps")
                            for k in range(NK):
                                nc.tensor.matmul(ps, lhsT=wh_sb[:, k, msl],
                                                 rhs=xsh[:, k, nsl],
                                                 start=(k == 0), stop=False)
                            for k in range(NK):
                                nc.tensor.matmul(ps, lhsT=wl_sb[:, k, msl],
                                                 rhs=xsh[:, k, nsl],
                                                 start=False, stop=False)
                            for k in range(NK):
                                nc.tensor.matmul(ps, lhsT=wh_sb[:, k, msl],
                                                 rhs=xsl[:, k, nsl],
                                                 start=False, stop=(k == NK - 1))
                            nc.vector.tensor_copy(sh_t[m][:, nsl], ps)
                            nc.vector.tensor_tensor(
                                sl_t[m][:, nsl], ps, sh_t[m][:, nsl], op=SUB)
                    for m in range(NK):
                        msl = slice(m * 128, (m + 1) * 128)
                        nc.sync.dma_start(outh[msl, :], sh_t[m])
                        nc.sync.dma_start(outl[msl, :], sl_t[m])
                if pfx == "q":
                    v_section()
    nc.compile()
    return nc


def _build_phase2():
    """Attention + output projection for this core's 1024 queries."""
    import concourse.mybir as mybir
    from concourse import bacc
    from concourse.tile import TileContext
    from concourse.masks import make_identity

    FP32 = mybir.dt.float32
    DBF = mybir.dt.bfloat16
    Exp = mybir.ActivationFunctionType.Exp
    AX = mybir.AxisListType.X

    nc = bacc.Bacc("TRN2", target_bir_lowering=False, debug=False, num_devices=8)

    kth = nc.dram_tensor("kth", [D, S], DBF, kind="ExternalInput")
    ktl = nc.dram_tensor("ktl", [D, S], DBF, kind="ExternalInput")
    # per-i-tile partition-major Q hi/lo: [i, p, hl, n, f] = qt{hl}[n*128+p, i*128+f]
    qt2 = nc.dram_tensor("qt2", [NI, 128, 2, NK, 128], DBF, kind="ExternalInput")
    vin = nc.dram_tensor("vin", [NT, 128, D], DBF, kind="ExternalInput")
    # per-m-chunk partition-major Wo: [m, p, n, f] = Wo[n*128+p, m*128+f]
    wo = nc.dram_tensor("wo", [NK, 128, NK, 128], DBF, kind="ExternalInput")
    yt = nc.dram_tensor("yt", [D, BLK], FP32, kind="ExternalOutput")

    def split_mms(ps, lh, ll, rh, rl, first, last):
        # rl (the lo half of the moving operand) is consumed last so its
        # DMA can land while the two rh terms compute
        nc.tensor.matmul(ps, lhsT=lh, rhs=rh, start=first, stop=False)
        nc.tensor.matmul(ps, lhsT=ll, rhs=rh, start=False, stop=False)
        nc.tensor.matmul(ps, lhsT=lh, rhs=rl, start=False, stop=last)

    from contextlib import ExitStack
    with TileContext(nc) as tc:
        with ExitStack() as stack:
            constp = stack.enter_context(tc.tile_pool(name="const", bufs=1))
            ident = constp.tile([128, 128], DBF)
            make_identity(nc, ident)

            ktp = stack.enter_context(tc.tile_pool(name="ktp", bufs=1))
            qtp = stack.enter_context(tc.tile_pool(name="qtp", bufs=2))
            if True:
                # first two i-tiles' Q loads before the big K transfers
                qv0 = qtp.tile([128, 2, NK, 128], DBF, name="qv0", tag="qv")
                nc.sync.dma_start(qv0, qt2[0])
                qv1 = qtp.tile([128, 2, NK, 128], DBF, name="qv1", tag="qv")
                nc.sync.dma_start(qv1, qt2[1])
                # column-chunked loads (2 j-blocks per DMA) so E work can
                # start as soon as the first chunks land; hi before lo
                kth_sb, ktl_sb = [], []
                for m in range(NK):
                    kth_sb.append(ktp.tile([128, S], DBF, name=f"kth{m}", tag=f"kth{m}"))
                    ktl_sb.append(ktp.tile([128, S], DBF, name=f"ktl{m}", tag=f"ktl{m}"))
                for cb in range(NJB // 2):
                    csl = slice(cb * 1024, (cb + 1) * 1024)
                    for m in range(NK):
                        nc.sync.dma_start(kth_sb[m][:, csl], kth[m * 128:(m + 1) * 128, csl])
                    for m in range(NK):
                        nc.sync.dma_start(ktl_sb[m][:, csl], ktl[m * 128:(m + 1) * 128, csl])

                otp = stack.enter_context(tc.tile_pool(name="otp", bufs=1))
                if True:
                    ot_sb = []
                    for t in range(NK):
                        ot_sb.append(otp.tile([128, BLK], DBF, name=f"ot{t}", tag=f"ot{t}"))
                    epsp = stack.enter_context(tc.tile_pool(name="eps", bufs=EPS_BUFS, space="PSUM"))
                    tpsp = stack.enter_context(tc.tile_pool(name="tps", bufs=TP_BUFS, space="PSUM"))
                    opsp = stack.enter_context(tc.tile_pool(name="ops", bufs=1, space="PSUM"))
                    smp = stack.enter_context(tc.tile_pool(name="smp", bufs=2))
                    esp = stack.enter_context(tc.tile_pool(name="esp", bufs=1))
                    pp = stack.enter_context(tc.tile_pool(name="pp", bufs=1))
                    ptp = stack.enter_context(tc.tile_pool(name="ptp", bufs=1))
                    vvp = stack.enter_context(tc.tile_pool(name="vvp", bufs=VV_BUFS))
                    obp = stack.enter_context(tc.tile_pool(name="obp", bufs=1))
                    wop = stack.enter_context(tc.tile_pool(name="wop", bufs=2))
                    ypsp = stack.enter_context(tc.tile_pool(name="yps", bufs=1, space="PSUM"))
                    ystp = stack.enter_context(tc.tile_pool(name="yst", bufs=2))
                    if True:
                        vin_r = vin[:, :, :].rearrange("(g w) p d -> p g w d", w=VG)

                        def softmax_pt(i):
                            """E + softmax + P^T for one i-tile; returns (pt, linv)."""
                            isl = slice(i * 128, (i + 1) * 128)
                            if i == 0:
                                qv = qv0
                            elif i == 1:
                                qv = qv1
                            else:
                                qv = qtp.tile([128, 2, NK, 128], DBF, name=f"qv{i}", tag="qv")
                                nc.sync.dma_start(qv, qt2[i])
                            mx8 = smp.tile([128, NJB], FP32, name=f"mx8_{i}", tag="mx8")
                            e_sb = esp.tile([128, S], FP32, name=f"e{i}", tag="e")
                            for jb in range(NJB):
                                sl = slice(jb * 512, (jb + 1) * 512)
                                ps = epsp.tile([128, 512], FP32, name=f"eps{i}_{jb}", tag="eps")
                                for k in range(NK):
                                    nc.tensor.matmul(ps, lhsT=qv[:, 0, k, :],
                                                     rhs=kth_sb[k][:, sl],
                                                     start=(k == 0), stop=False)
                                    nc.tensor.matmul(ps, lhsT=qv[:, 1, k, :],
                                                     rhs=kth_sb[k][:, sl],
                                                     start=False, stop=False)
                                for k in range(NK):
                                    nc.tensor.matmul(ps, lhsT=qv[:, 0, k, :],
                                                     rhs=ktl_sb[k][:, sl],
                                                     start=False, stop=(k == NK - 1))
                                nc.vector.tensor_copy(e_sb[:, sl], ps)
                                nc.vector.reduce_max(mx8[:, jb:jb + 1], ps, axis=AX)

                            mrow = smp.tile([128, 1], FP32, name=f"mrow{i}", tag="mrow")
                            nc.vector.reduce_max(mrow, mx8, axis=AX)
                            negm = smp.tile([128, 1], FP32, name=f"negm{i}", tag="negm")
                            nc.vector.tensor_scalar_mul(negm, mrow, -SCALE)

                            p_sb = pp.tile([128, S], DBF, name=f"p{i}", tag="p")
                            lp8 = smp.tile([128, NJB], FP32, name=f"lp8_{i}", tag="lp8")
                            for jb in range(NJB):
                                sl = slice(jb * 512, (jb + 1) * 512)
                                nc.scalar.activation(
                                    p_sb[:, sl], e_sb[:, sl], Exp,
                                    bias=negm, scale=SCALE,
                                    accum_out=lp8[:, jb:jb + 1],
                                )
                            lrow = smp.tile([128, 1], FP32, name=f"lrow{i}", tag="lrow")
                            nc.vector.reduce_sum(lrow, lp8, axis=AX)
                            linv = smp.tile([128, 1], FP32, name=f"linv{i}", tag="linv")
                            nc.vector.reciprocal(linv, lrow)

                            pt_sb = ptp.tile([128, NT, 128], DBF, name=f"pt{i}", tag="pt")
                            for t in range(NT):
                                tp = tpsp.tile([128, 128], DBF, name=f"tp{i}_{t}", tag="tp")
                                nc.tensor.transpose(tp, p_sb[:, t * 128:(t + 1) * 128], ident)
                                nc.vector.tensor_copy(pt_sb[:, t, :], tp)
                            return pt_sb, linv

                        def y_cols(off, w, pre=()):
                            # Y = O @ Wo for output columns [off, off+w)
                            # (needs i-tiles off/128 .. (off+w)/128-1 finished)
                            nsl = slice(off, off + w)
                            for m in range(NK):
                                if m < len(pre):
                                    wom = pre[m]
                                else:
                                    # stream this m-chunk of Wo
                                    wom = wop.tile([128, NK, 128], DBF, name=f"wo{off}_{m}", tag="wom")
                                    nc.sync.dma_start(wom, wo[m])
                                ps = ypsp.tile([128, 512], FP32, name=f"yps{m}_{off}", tag="yps")
                                for k in range(NK):
                                    nc.tensor.matmul(
                                        ps[:, 0:w], lhsT=wom[:, k, :],
                                        rhs=ot_sb[k][:, nsl],
                                        start=(k == 0), stop=(k == NK - 1),
                                    )
                                ys = ystp.tile([128, 512], FP32, name=f"ys{m}_{off}", tag="ys")
                                nc.vector.tensor_copy(ys[:, 0:w], ps[:, 0:w])
                                nc.sync.dma_start(yt[m * 128:(m + 1) * 128, nsl], ys[:, 0:w])

                        def o_finish(i, op0, op1, linv):
                            """1/l scale + O transpose into ot_sb for one i-tile."""
                            isl = slice(i * 128, (i + 1) * 128)
                            osb = obp.tile([128, D], DBF, name=f"osb{i}", tag="osb")
                            nc.vector.tensor_scalar_mul(osb[:, 0:512], op0, linv)
                            nc.vector.tensor_scalar_mul(osb[:, 512:D], op1, linv)
                            for t in range(NK):
                                tp = tpsp.tile([128, 128], DBF, name=f"otp{i}_{t}", tag="tp")
                                nc.tensor.transpose(tp, osb[:, t * 128:(t + 1) * 128], ident)
                                nc.vector.tensor_copy(ot_sb[t][:, isl], tp)

                        if True:
                            pre7 = []
                            for i in range(NI):
                                if i == 4:
                                    y_cols(0, 512)
                                if i == NI - 1:
                                    # prefetch the first Wo chunks of the
                                    # final Y half ahead of the last V sweep
                                    for m in range(2):
                                        wom = wop.tile([128, NK, 128], DBF,
                                                       name=f"wop1_{m}", tag="wom")
                                        nc.sync.dma_start(wom, wo[m])
                                        pre7.append(wom)
                                pt_sb, linv = softmax_pt(i)
                                op0 = opsp.tile([128, 512], FP32, name=f"op0_{i}", tag="op0")
                                op1 = opsp.tile([128, 512], FP32, name=f"op1_{i}", tag="op1")
                                for g in range(NT // VG):
                                    vv = vvp.tile([128, VG, D], DBF, name=f"vv{i}_{g}", tag="vv")
                                    nc.sync.dma_start(vv, vin_r[:, g])
                                    for w in range(VG):
                                        t = g * VG + w
                                        nc.tensor.matmul(
                                            op0, lhsT=pt_sb[:, t, :], rhs=vv[:, w, 0:512],
                                            start=(t == 0), stop=(t == NT - 1))
                                        nc.tensor.matmul(
                                            op1, lhsT=pt_sb[:, t, :], rhs=vv[:, w, 512:D],
                                            start=(t == 0), stop=(t == NT - 1))
                                o_finish(i, op0, op1, linv)
                            y_cols(512, 512, pre=pre7)
    nc.compile()
    return nc


def _get_programs():
    if "nc1" not in _cache:
        _cache["nc1"] = _build_phase1()
        _cache["nc2"] = _build_phase2()
    return _cache["nc1"], _cache["nc2"]


def kernel(x, Wq, Wk, Wv, Wo):
    from concourse.bass_utils import run_bass_kernel_spmd

    nc1, nc2 = _get_programs()

    x = np.asarray(x, dtype=np.float32)
    wqh, wql = _split_hilo(np.asarray(Wq, dtype=np.float32))
    wkh, wkl = _split_hilo(np.asarray(Wk, dtype=np.float32))
    wv_b = np.asarray(Wv, dtype=np.float32).astype(BF16)
    wo_b = np.asarray(Wo, dtype=np.float32).astype(BF16)
    wo_blk = np.ascontiguousarray(
        wo_b.reshape(NK, 128, NK, 128).transpose(2, 1, 0, 3))

    # ---- phase 1: per-core row slices ----
    in1 = []
    for c in range(8):
        b, i = divmod(c, 4)
        rows = x[b, i * BLK:(i + 1) * BLK, :]           # [BLK, D]
        xh, xl = _split_hilo(np.ascontiguousarray(rows.T))
        xb = np.ascontiguousarray(
            rows.astype(BF16).reshape(NI, 128, NK, 128).transpose(0, 3, 2, 1))
        in1.append({
            "xh": xh, "xl": xl, "xb": xb,
            "wqh": wqh, "wql": wql, "wkh": wkh, "wkl": wkl, "wv": wv_b,
        })
    res1 = run_bass_kernel_spmd(nc1, in1, list(range(8))).results

    # ---- host gather of K/V shards into per-batch tensors ----
    kth_full, ktl_full, v_full = [], [], []
    for b in range(B):
        kth_full.append(np.concatenate(
            [res1[b * 4 + i]["kth"] for i in range(4)], axis=1))   # [D, S]
        ktl_full.append(np.concatenate(
            [res1[b * 4 + i]["ktl"] for i in range(4)], axis=1))
        v_full.append(np.concatenate(
            [res1[b * 4 + i]["vo"] for i in range(4)], axis=0))    # [NT, 128, D]

    # ---- phase 2 ----
    in2 = []
    for c in range(8):
        b, i = divmod(c, 4)
        qstack = np.stack([
            res1[c]["qth"].reshape(NK, 128, NI, 128),
            res1[c]["qtl"].reshape(NK, 128, NI, 128)], axis=0)  # [hl, n, p, i, f]
        in2.append({
            "kth": kth_full[b], "ktl": ktl_full[b], "vin": v_full[b],
            "qt2": np.ascontiguousarray(qstack.transpose(3, 2, 0, 1, 4)),
            "wo": wo_blk,
        })
    res2 = run_bass_kernel_spmd(nc2, in2, list(range(8))).results

    out = np.empty((B, S, D), dtype=np.float32)
    for c in range(8):
        b, i = divmod(c, 4)
        out[b, i * BLK:(i + 1) * BLK, :] = res2[c]["yt"].T
    return out



# revision 2
# speedup vs baseline: 1.6434x; 1.6434x over previous
"""Trainium2 Bass kernel for single-head self-attention (B=2, S=4096, D=1024).

reference:
    q = x @ Wq; k = x @ Wk; v = x @ Wv          # [B,S,D]
    energy = einsum('bid,bjd->bij', q, k) * 8.0  # SCALE = sqrt(64)
    attn = softmax(energy, axis=-1)
    out = einsum('bij,bjd->bid', attn, v) @ Wo

Two SPMD launches over 8 cores (= 2 batches x 4 query-blocks of 1024):
  phase 1: each core computes the Q/K/V projections for its own 1024
           rows only (1/8 of the total work, no redundancy); the host
           gathers K/V shards into full per-batch tensors.
  phase 2: each core runs attention + output projection for its block
           against the full K/V of its batch.

Precision: logits have std ~256 (SCALE multiplies), so softmax is
nearly an argmax -- the x->Q, x->K, Q@K^T path needs much better than
bf16 input precision.  The PE's fp32r mode runs single-pass at the
bf16 rate (1 cycle/row for moving free-dim >= 256) with ~13 effective
mantissa bits (measured rel err 1.5e-4 on a K=1024 contraction), which
puts ~0.07 std of noise on the logits -> ~4e-3 output rel err.  The
whole Q/K path (both projections and Q@K^T) therefore runs fp32r
single-pass; V / P / Wo run in bf16.

Layout: feature-major ("transposed") activations throughout; the host
pre-transposes x and post-transposes the output. DMA instruction count
is kept low (batched loads/stores) -- each HWDGE descriptor-generation
costs ~0.6us of serialized queue-prep time.
"""

import numpy as np
import ml_dtypes

B, S, D = 2, 4096, 1024
BLK = 1024          # queries per core
SCALE = 8.0         # HEAD_DIM ** 0.5 = sqrt(64)
NK = D // 128       # 8 k-tiles over the feature dim
NT = S // 128       # 32 j-tiles over keys
NI = BLK // 128     # 8 i-tiles over this core's queries
NJB = S // 512      # 8 key blocks of 512
BF16 = ml_dtypes.bfloat16

# phase-2 tuning knobs
EPS_BUFS = 3        # PSUM banks for E accumulation
TP_BUFS = 2         # PSUM banks for PE transposes
VG = 1              # V j-tiles per DMA
VV_BUFS = 6

_cache = {}


def _build_phase1():
    """Q/K/V projections for this core's 1024 rows (all fp32r single-pass)."""
    import concourse.mybir as mybir
    from concourse import bacc
    from concourse.tile import TileContext

    FP32 = mybir.dt.float32
    FP32R = mybir.dt.float32r
    DBF = mybir.dt.bfloat16

    nc = bacc.Bacc("TRN2", target_bir_lowering=False, debug=False, num_devices=8)

    xt = nc.dram_tensor("xt", [D, BLK], FP32R, kind="ExternalInput")  # rows.T
    wq = nc.dram_tensor("wq", [D, D], FP32R, kind="ExternalInput")
    wk = nc.dram_tensor("wk", [D, D], FP32R, kind="ExternalInput")
    wv = nc.dram_tensor("wv", [D, D], FP32R, kind="ExternalInput")
    qt = nc.dram_tensor("qt", [D, BLK], FP32, kind="ExternalOutput")
    kt = nc.dram_tensor("kt", [D, BLK], FP32, kind="ExternalOutput")
    vo = nc.dram_tensor("vo", [NI, 128, D], DBF, kind="ExternalOutput")

    with TileContext(nc) as tc:
      with (
          tc.tile_pool(name="xp", bufs=1) as xp,
          tc.tile_pool(name="wp", bufs=1) as wp,
          tc.tile_pool(name="vps", bufs=2, space="PSUM") as vps,
          tc.tile_pool(name="vsb", bufs=3) as vsbp,
      ):
        # load order = consumption order; x/K-weight halves first so the
        # first K sweep can start early
        xt_r = xt[:, :].rearrange("(n p) s -> p n s", p=128)
        x_sb = xp.tile([128, NK, BLK], FP32R, name="x_sb", tag="x_sb")
        wk_sb = wp.tile([128, NK, D], FP32R, name="wk_sb", tag="wk_sb")
        wk_r = wk[:, :].rearrange("(n p) d -> p n d", p=128)
        nc.sync.dma_start(x_sb[:, :, 0:512], xt_r[:, :, 0:512])
        nc.sync.dma_start(wk_sb[:, :, 0:256], wk_r[:, :, 0:256])
        nc.sync.dma_start(wk_sb[:, :, 256:512], wk_r[:, :, 256:512])
        nc.sync.dma_start(x_sb[:, :, 512:BLK], xt_r[:, :, 512:BLK])
        nc.sync.dma_start(wk_sb[:, :, 512:D], wk_r[:, :, 512:D])
        wq_sb = wp.tile([128, NK, D], FP32R, name="wq_sb", tag="wq_sb")
        wq_r = wq[:, :].rearrange("(n p) d -> p n d", p=128)
        nc.sync.dma_start(wq_sb[:, :, 0:512], wq_r[:, :, 0:512])
        nc.sync.dma_start(wq_sb[:, :, 512:D], wq_r[:, :, 512:D])
        wv_sb = wp.tile([128, NK, D], FP32R, name="wv_sb", tag="wv_sb")
        wv_r = wv[:, :].rearrange("(n p) d -> p n d", p=128)
        nc.sync.dma_start(wv_sb[:, :, 0:512], wv_r[:, :, 0:512])
        nc.sync.dma_start(wv_sb[:, :, 512:D], wv_r[:, :, 512:D])

        # ------------- Kt, then Qt, then V -------------
        for (wsb, outd, pfx) in ((wk_sb, kt, "k"), (wq_sb, qt, "q")):
            with (
                tc.tile_pool(name=f"{pfx}ps", bufs=4, space="PSUM") as psp,
                tc.tile_pool(name=f"{pfx}st", bufs=1) as stp,
            ):
                st = []
                for m in range(NK):
                    st.append(stp.tile([128, BLK], FP32, name=f"{pfx}s{m}",
                                       tag=f"s{m}"))
                for nb in range(2):
                    nsl = slice(nb * 512, (nb + 1) * 512)
                    for m in range(NK):
                        msl = slice(m * 128, (m + 1) * 128)
                        ps = psp.tile([128, 512], FP32, name=f"{pfx}ps{nb}_{m}",
                                      tag="ps")
                        for k in range(NK):
                            nc.tensor.matmul(ps, lhsT=wsb[:, k, msl],
                                             rhs=x_sb[:, k, nsl],
                                             start=(k == 0), stop=(k == NK - 1))
                        nc.vector.tensor_copy(st[m][:, nsl], ps)
                for m in range(NK):
                    msl = slice(m * 128, (m + 1) * 128)
                    nc.sync.dma_start(outd[msl, :], st[m])

        for j in range(NI):
            jsl = slice(j * 128, (j + 1) * 128)
            vt = vsbp.tile([128, D], DBF, name=f"vt{j}", tag="vt")
            for db in range(2):
                ps = vps.tile([128, 512], FP32, name=f"vps{j}_{db}", tag="vps")
                for k in range(NK):
                    nc.tensor.matmul(
                        ps, lhsT=x_sb[:, k, jsl],
                        rhs=wv_sb[:, k, db * 512:(db + 1) * 512],
                        start=(k == 0), stop=(k == NK - 1),
                    )
                nc.vector.tensor_copy(vt[:, db * 512:(db + 1) * 512], ps)
            nc.sync.dma_start(vo[j], vt)
    nc.compile()
    return nc


def _build_phase2():
    """Attention + output projection for this core's 1024 queries."""
    import concourse.mybir as mybir
    from concourse import bacc
    from concourse.tile import TileContext
    from concourse.masks import make_identity

    FP32 = mybir.dt.float32
    FP32R = mybir.dt.float32r
    DBF = mybir.dt.bfloat16
    Exp = mybir.ActivationFunctionType.Exp
    AX = mybir.AxisListType.X

    nc = bacc.Bacc("TRN2", target_bir_lowering=False, debug=False, num_devices=8)

    kth = nc.dram_tensor("kth", [D, S], FP32R, kind="ExternalInput")
    # per-i-tile partition-major Q: [i, p, n, f] = qt[n*128+p, i*128+f]
    qt2 = nc.dram_tensor("qt2", [NI, 128, NK, 128], FP32R, kind="ExternalInput")
    vin = nc.dram_tensor("vin", [NT, 128, D], DBF, kind="ExternalInput")
    # per-m-chunk partition-major Wo: [m, p, n, f] = Wo[n*128+p, m*128+f]
    wo = nc.dram_tensor("wo", [NK, 128, NK, 128], DBF, kind="ExternalInput")
    yt = nc.dram_tensor("yt", [D, BLK], FP32, kind="ExternalOutput")

    from contextlib import ExitStack
    with TileContext(nc) as tc:
        with ExitStack() as stack:
            constp = stack.enter_context(tc.tile_pool(name="const", bufs=1))
            ident = constp.tile([128, 128], DBF)
            make_identity(nc, ident)

            ktp = stack.enter_context(tc.tile_pool(name="ktp", bufs=1))
            qtp = stack.enter_context(tc.tile_pool(name="qtp", bufs=2))
            # first two i-tiles' Q loads before the big K transfers
            qv0 = qtp.tile([128, NK, 128], FP32R, name="qv0", tag="qv")
            nc.sync.dma_start(qv0, qt2[0])
            qv1 = qtp.tile([128, NK, 128], FP32R, name="qv1", tag="qv")
            nc.sync.dma_start(qv1, qt2[1])
            # column-chunked loads (2 j-blocks per DMA) so E work can
            # start as soon as the first chunks land
            kth_sb = []
            for m in range(NK):
                kth_sb.append(ktp.tile([128, S], FP32R, name=f"kth{m}",
                                       tag=f"kth{m}"))
            for cb in range(NJB // 2):
                csl = slice(cb * 1024, (cb + 1) * 1024)
                for m in range(NK):
                    nc.sync.dma_start(kth_sb[m][:, csl],
                                      kth[m * 128:(m + 1) * 128, csl])

            otp = stack.enter_context(tc.tile_pool(name="otp", bufs=1))
            ot_sb = []
            for t in range(NK):
                ot_sb.append(otp.tile([128, BLK], DBF, name=f"ot{t}", tag=f"ot{t}"))
            epsp = stack.enter_context(tc.tile_pool(name="eps", bufs=EPS_BUFS, space="PSUM"))
            tpsp = stack.enter_context(tc.tile_pool(name="tps", bufs=TP_BUFS, space="PSUM"))
            opsp = stack.enter_context(tc.tile_pool(name="ops", bufs=1, space="PSUM"))
            smp = stack.enter_context(tc.tile_pool(name="smp", bufs=2))
            esp = stack.enter_context(tc.tile_pool(name="esp", bufs=1))
            pp = stack.enter_context(tc.tile_pool(name="pp", bufs=1))
            ptp = stack.enter_context(tc.tile_pool(name="ptp", bufs=1))
            vvp = stack.enter_context(tc.tile_pool(name="vvp", bufs=VV_BUFS))
            obp = stack.enter_context(tc.tile_pool(name="obp", bufs=1))
            wop = stack.enter_context(tc.tile_pool(name="wop", bufs=2))
            ypsp = stack.enter_context(tc.tile_pool(name="yps", bufs=1, space="PSUM"))
            ystp = stack.enter_context(tc.tile_pool(name="yst", bufs=2))

            vin_r = vin[:, :, :].rearrange("(g w) p d -> p g w d", w=VG)

            def softmax_pt(i):
                """E + softmax + P^T for one i-tile; returns (pt, linv)."""
                if i == 0:
                    qv = qv0
                elif i == 1:
                    qv = qv1
                else:
                    qv = qtp.tile([128, NK, 128], FP32R, name=f"qv{i}", tag="qv")
                    nc.sync.dma_start(qv, qt2[i])
                mx8 = smp.tile([128, NJB], FP32, name=f"mx8_{i}", tag="mx8")
                e_sb = esp.tile([128, S], FP32, name=f"e{i}", tag="e")
                for jb in range(NJB):
                    sl = slice(jb * 512, (jb + 1) * 512)
                    ps = epsp.tile([128, 512], FP32, name=f"eps{i}_{jb}", tag="eps")
                    for k in range(NK):
                        nc.tensor.matmul(ps, lhsT=qv[:, k, :],
                                         rhs=kth_sb[k][:, sl],
                                         start=(k == 0), stop=(k == NK - 1))
                    nc.vector.tensor_copy(e_sb[:, sl], ps)
                    nc.vector.reduce_max(mx8[:, jb:jb + 1], ps, axis=AX)

                mrow = smp.tile([128, 1], FP32, name=f"mrow{i}", tag="mrow")
                nc.vector.reduce_max(mrow, mx8, axis=AX)
                negm = smp.tile([128, 1], FP32, name=f"negm{i}", tag="negm")
                nc.vector.tensor_scalar_mul(negm, mrow, -SCALE)

                p_sb = pp.tile([128, S], DBF, name=f"p{i}", tag="p")
                lp8 = smp.tile([128, NJB], FP32, name=f"lp8_{i}", tag="lp8")
                for jb in range(NJB):
                    sl = slice(jb * 512, (jb + 1) * 512)
                    nc.scalar.activation(
                        p_sb[:, sl], e_sb[:, sl], Exp,
                        bias=negm, scale=SCALE,
                        accum_out=lp8[:, jb:jb + 1],
                    )
                lrow = smp.tile([128, 1], FP32, name=f"lrow{i}", tag="lrow")
                nc.vector.reduce_sum(lrow, lp8, axis=AX)
                linv = smp.tile([128, 1], FP32, name=f"linv{i}", tag="linv")
                nc.vector.reciprocal(linv, lrow)

                pt_sb = ptp.tile([128, NT, 128], DBF, name=f"pt{i}", tag="pt")
                for t in range(NT):
                    tp = tpsp.tile([128, 128], DBF, name=f"tp{i}_{t}", tag="tp")
                    nc.tensor.transpose(tp, p_sb[:, t * 128:(t + 1) * 128], ident)
                    nc.vector.tensor_copy(pt_sb[:, t, :], tp)
                return pt_sb, linv

            def y_cols(off, w, pre=()):
                # Y = O @ Wo for output columns [off, off+w)
                # (needs i-tiles off/128 .. (off+w)/128-1 finished)
                nsl = slice(off, off + w)
                for m in range(NK):
                    if m < len(pre):
                        wom = pre[m]
                    else:
                        wom = wop.tile([128, NK, 128], DBF, name=f"wo{off}_{m}",
                                       tag="wom")
                        nc.sync.dma_start(wom, wo[m])
                    ps = ypsp.tile([128, 512], FP32, name=f"yps{m}_{off}", tag="yps")
                    for k in range(NK):
                        nc.tensor.matmul(
                            ps[:, 0:w], lhsT=wom[:, k, :],
                            rhs=ot_sb[k][:, nsl],
                            start=(k == 0), stop=(k == NK - 1),
                        )
                    ys = ystp.tile([128, 512], FP32, name=f"ys{m}_{off}", tag="ys")
                    nc.vector.tensor_copy(ys[:, 0:w], ps[:, 0:w])
                    nc.sync.dma_start(yt[m * 128:(m + 1) * 128, nsl], ys[:, 0:w])

            def o_finish(i, op0, op1, linv):
                """1/l scale + O transpose into ot_sb for one i-tile."""
                isl = slice(i * 128, (i + 1) * 128)
                osb = obp.tile([128, D], DBF, name=f"osb{i}", tag="osb")
                nc.vector.tensor_scalar_mul(osb[:, 0:512], op0, linv)
                nc.vector.tensor_scalar_mul(osb[:, 512:D], op1, linv)
                for t in range(NK):
                    tp = tpsp.tile([128, 128], DBF, name=f"otp{i}_{t}", tag="tp")
                    nc.tensor.transpose(tp, osb[:, t * 128:(t + 1) * 128], ident)
                    nc.vector.tensor_copy(ot_sb[t][:, isl], tp)

            pre7 = []
            for i in range(NI):
                if i == 4:
                    y_cols(0, 512)
                if i == NI - 1:
                    # prefetch the first Wo chunks of the final Y half
                    # ahead of the last V sweep
                    for m in range(2):
                        wom = wop.tile([128, NK, 128], DBF,
                                       name=f"wop1_{m}", tag="wom")
                        nc.sync.dma_start(wom, wo[m])
                        pre7.append(wom)
                pt_sb, linv = softmax_pt(i)
                op0 = opsp.tile([128, 512], FP32, name=f"op0_{i}", tag="op0")
                op1 = opsp.tile([128, 512], FP32, name=f"op1_{i}", tag="op1")
                for g in range(NT // VG):
                    vv = vvp.tile([128, VG, D], DBF, name=f"vv{i}_{g}", tag="vv")
                    nc.sync.dma_start(vv, vin_r[:, g])
                    for w in range(VG):
                        t = g * VG + w
                        nc.tensor.matmul(
                            op0, lhsT=pt_sb[:, t, :], rhs=vv[:, w, 0:512],
                            start=(t == 0), stop=(t == NT - 1))
                        nc.tensor.matmul(
                            op1, lhsT=pt_sb[:, t, :], rhs=vv[:, w, 512:D],
                            start=(t == 0), stop=(t == NT - 1))
                o_finish(i, op0, op1, linv)
            y_cols(512, 512, pre=pre7)
    nc.compile()
    return nc


def _get_programs():
    if "nc1" not in _cache:
        _cache["nc1"] = _build_phase1()
        _cache["nc2"] = _build_phase2()
    return _cache["nc1"], _cache["nc2"]


def kernel(x, Wq, Wk, Wv, Wo):
    from concourse.bass_utils import run_bass_kernel_spmd

    nc1, nc2 = _get_programs()

    x = np.asarray(x, dtype=np.float32)
    wq_f = np.asarray(Wq, dtype=np.float32)
    wk_f = np.asarray(Wk, dtype=np.float32)
    wv_f = np.asarray(Wv, dtype=np.float32)
    wo_b = np.asarray(Wo, dtype=np.float32).astype(BF16)
    wo_blk = np.ascontiguousarray(
        wo_b.reshape(NK, 128, NK, 128).transpose(2, 1, 0, 3))

    # ---- phase 1: per-core row slices ----
    in1 = []
    for c in range(8):
        b, i = divmod(c, 4)
        rows = x[b, i * BLK:(i + 1) * BLK, :]           # [BLK, D]
        in1.append({
            "xt": np.ascontiguousarray(rows.T),
            "wq": wq_f, "wk": wk_f, "wv": wv_f,
        })
    res1 = run_bass_kernel_spmd(nc1, in1, list(range(8))).results

    # ---- host gather of K/V shards into per-batch tensors ----
    kt_full, v_full = [], []
    for b in range(B):
        kt_full.append(np.concatenate(
            [res1[b * 4 + i]["kt"] for i in range(4)], axis=1))   # [D, S]
        v_full.append(np.concatenate(
            [res1[b * 4 + i]["vo"] for i in range(4)], axis=0))   # [NT, 128, D]

    # ---- phase 2 ----
    in2 = []
    for c in range(8):
        b, i = divmod(c, 4)
        qstack = res1[c]["qt"].reshape(NK, 128, NI, 128)  # [n, p, i, f]
        in2.append({
            "kth": kt_full[b], "vin": v_full[b],
            "qt2": np.ascontiguousarray(qstack.transpose(2, 1, 0, 3)),
            "wo": wo_blk,
        })
    res2 = run_bass_kernel_spmd(nc2, in2, list(range(8))).results

    out = np.empty((B, S, D), dtype=np.float32)
    for c in range(8):
        b, i = divmod(c, 4)
        out[b, i * BLK:(i + 1) * BLK, :] = res2[c]["yt"].T
    return out


# revision 25
# speedup vs baseline: 2.0811x; 1.2663x over previous
"""Trainium2 Bass kernel for single-head self-attention (B=2, S=4096, D=1024).

reference:
    q = x @ Wq; k = x @ Wk; v = x @ Wv          # [B,S,D]
    energy = einsum('bid,bjd->bij', q, k) * 8.0  # SCALE = sqrt(64)
    attn = softmax(energy, axis=-1)
    out = einsum('bij,bjd->bid', attn, v) @ Wo

Weight folding (associativity): energy = x @ (Wq Wk^T) @ x^T and
out = attn @ (x @ (Wv Wo)), so the host precomputes M = Wq @ Wk^T and
W' = Wv @ Wo once (fp64) and the device only runs TWO projections
(G = x@M, V' = x@W') plus the two S^2-sized attention matmuls -- the
separate K projection and the output projection disappear.

Two SPMD launches over 8 cores (= 2 batches x 4 query-blocks of 1024):
  phase 1: each core computes G / V' for its own 1024 rows; the host
           gathers V' shards (and pre-casts x^T to fp16) per batch.
  phase 2: each core computes softmax(G_blk @ x^T * 8) @ V' for its
           1024 queries against the full batch; output rows come out
           of the P@V' accumulation directly.

Precision: logits have std ~256 (SCALE multiplies), so the logit path
needs much better than bf16 input precision.  The G projection runs in
the PE's fp32r mode (single-pass at the bf16 rate for free-dim >= 256,
~13 effective mantissa bits); G and x^T are stored as fp16 (2^-11
rounding) and G @ x^T runs in fp16 (fp16 products are exact in fp32
PSUM accumulation).  Combined logit noise ~0.11 std -> ~6e-3 output
rel err.  V' / P run in bf16.

Phase-2 layout: x^T (fp16, 64KB/part) and V' (bf16, 64KB/part) are
SBUF-resident, so after the initial load the attention sweep runs with
no input DMA.  A dma_start occupies its issuing queue through the
whole transfer, so queue assignment is part of the schedule: bulk
loads ride SP in consumption order, the ACT queue stays clear for the
softmax exp chain, and E psum->SBUF copies + P^T copies run on Pool.
"""

import numpy as np
import ml_dtypes

B, S, D = 2, 4096, 1024
BLK = 1024          # queries per core
SCALE = 8.0         # HEAD_DIM ** 0.5 = sqrt(64)
NK = D // 128       # 8 k-tiles over the feature dim
NT = S // 128       # 32 j-tiles over keys
NI = BLK // 128     # 8 i-tiles over this core's queries
NJB = S // 512      # 8 key blocks of 512
BF16 = ml_dtypes.bfloat16

_cache = {}


def _build_phase1():
    """G = x@M and V' = x@W' for this core's 1024 rows (fp32r single-pass)."""
    import concourse.mybir as mybir
    from concourse import bacc
    from concourse.tile import TileContext

    FP16 = mybir.dt.float16
    FP32 = mybir.dt.float32
    FP32R = mybir.dt.float32r
    DBF = mybir.dt.bfloat16

    nc = bacc.Bacc("TRN2", target_bir_lowering=False, debug=False, num_devices=8)

    xt = nc.dram_tensor("xt", [D, BLK], FP32R, kind="ExternalInput")  # rows.T
    wm = nc.dram_tensor("wm", [D, D], FP32R, kind="ExternalInput")    # Wq@Wk^T
    wvo = nc.dram_tensor("wvo", [D, D], FP32R, kind="ExternalInput")  # Wv@Wo
    gt = nc.dram_tensor("gt", [D, BLK], FP16, kind="ExternalOutput")
    vo = nc.dram_tensor("vo", [NI, 128, D], DBF, kind="ExternalOutput")

    with TileContext(nc) as tc:
      with (
          tc.tile_pool(name="xp", bufs=1) as xp,
          tc.tile_pool(name="wp", bufs=1) as wp,
          tc.tile_pool(name="gps", bufs=4, space="PSUM") as gps,
          tc.tile_pool(name="gst", bufs=1) as gstp,
          tc.tile_pool(name="vps", bufs=2, space="PSUM") as vps,
          tc.tile_pool(name="vsb", bufs=3) as vsbp,
      ):
        # all loads on the SP queue in consumption order (the serialized
        # DMA engine then transfers them in priority order); first slices
        # small so the first G block can start ~6us in
        xt_r = xt[:, :].rearrange("(n p) s -> p n s", p=128)
        x_sb = xp.tile([128, NK, BLK], FP32R, name="x_sb", tag="x_sb")
        wm_sb = wp.tile([128, NK, D], FP32R, name="wm_sb", tag="wm_sb")
        wm_r = wm[:, :].rearrange("(n p) d -> p n d", p=128)
        nc.sync.dma_start(x_sb[:, :, 0:256], xt_r[:, :, 0:256])
        nc.sync.dma_start(wm_sb[:, :, 0:256], wm_r[:, :, 0:256])
        nc.sync.dma_start(x_sb[:, :, 256:512], xt_r[:, :, 256:512])
        nc.sync.dma_start(wm_sb[:, :, 256:512], wm_r[:, :, 256:512])
        nc.sync.dma_start(x_sb[:, :, 512:BLK], xt_r[:, :, 512:BLK])
        nc.sync.dma_start(wm_sb[:, :, 512:768], wm_r[:, :, 512:768])
        nc.sync.dma_start(wm_sb[:, :, 768:D], wm_r[:, :, 768:D])
        wv_sb = wp.tile([128, NK, D], FP32R, name="wv_sb", tag="wv_sb")
        wvo_r = wvo[:, :].rearrange("(n p) d -> p n d", p=128)
        nc.sync.dma_start(wv_sb[:, :, 0:512], wvo_r[:, :, 0:512])
        nc.sync.dma_start(wv_sb[:, :, 512:D], wvo_r[:, :, 512:D])

        # G blocks emitted in DMA-supply order: each group becomes runnable
        # as one more of the loads above lands.
        K_ORDER = [
            (0, 0), (0, 1),
            (1, 0), (1, 1),
            (0, 2), (0, 3), (1, 2), (1, 3),
            (2, 0), (2, 1), (2, 2), (2, 3),
            (0, 4), (0, 5), (1, 4), (1, 5), (2, 4), (2, 5),
            (0, 6), (0, 7), (1, 6), (1, 7), (2, 6), (2, 7),
        ]
        NBS = ((0, 256), (256, 256), (512, 512))
        st = []
        for m in range(NK):
            st.append(gstp.tile([128, BLK], FP16, name=f"gs{m}", tag=f"s{m}"))
        for (nb, m) in K_ORDER:
            n0, nw = NBS[nb]
            nsl = slice(n0, n0 + nw)
            msl = slice(m * 128, (m + 1) * 128)
            ps = gps.tile([128, 512], FP32, name=f"gp{n0}_{m}", tag="ps")
            for k in range(NK):
                nc.tensor.matmul(ps[:, 0:nw], lhsT=wm_sb[:, k, msl],
                                 rhs=x_sb[:, k, nsl],
                                 start=(k == 0), stop=(k == NK - 1))
            nc.vector.tensor_copy(st[m][:, nsl], ps[:, 0:nw])
        for m in range(NK):
            nc.sync.dma_start(gt[m * 128:(m + 1) * 128, :], st[m])

        for j in range(NI):
            jsl = slice(j * 128, (j + 1) * 128)
            vt = vsbp.tile([128, D], DBF, name=f"vt{j}", tag="vt")
            for db in range(2):
                ps = vps.tile([128, 512], FP32, name=f"vps{j}_{db}", tag="vps")
                for k in range(NK):
                    nc.tensor.matmul(
                        ps, lhsT=x_sb[:, k, jsl],
                        rhs=wv_sb[:, k, db * 512:(db + 1) * 512],
                        start=(k == 0), stop=(k == NK - 1),
                    )
                nc.vector.tensor_copy(vt[:, db * 512:(db + 1) * 512], ps)
                nc.scalar.dma_start(vo[j][:, db * 512:(db + 1) * 512],
                                    vt[:, db * 512:(db + 1) * 512])
    nc.compile()
    return nc


def _build_phase2():
    """softmax(G_blk @ x^T * 8) @ V' for this core's 1024 queries."""
    import concourse.mybir as mybir
    from concourse import bacc
    from concourse.tile import TileContext
    from concourse.masks import make_identity

    FP16 = mybir.dt.float16
    FP32 = mybir.dt.float32
    DBF = mybir.dt.bfloat16
    Exp = mybir.ActivationFunctionType.Exp
    Copy = mybir.ActivationFunctionType.Copy
    AX = mybir.AxisListType.X

    nc = bacc.Bacc("TRN2", target_bir_lowering=False, debug=False, num_devices=8)

    xth = nc.dram_tensor("xth", [D, S], FP16, kind="ExternalInput")
    # per-i-tile partition-major G: [i, p, n, f] = gt[n*128+p, i*128+f]
    gt2 = nc.dram_tensor("gt2", [NI, 128, NK, 128], FP16, kind="ExternalInput")
    # partition-major V': [p, t, d] = V'[t*128+p, d]
    vin = nc.dram_tensor("vin", [128, NT, D], DBF, kind="ExternalInput")
    y = nc.dram_tensor("y", [BLK, D], FP16, kind="ExternalOutput")

    from contextlib import ExitStack
    with TileContext(nc) as tc:
        with ExitStack() as stack:
            constp = stack.enter_context(tc.tile_pool(name="const", bufs=1))
            ident = constp.tile([128, 128], DBF)
            make_identity(nc, ident)

            ktp = stack.enter_context(tc.tile_pool(name="ktp", bufs=1))
            qtp = stack.enter_context(tc.tile_pool(name="qtp", bufs=4))
            vvp = stack.enter_context(tc.tile_pool(name="vvp", bufs=1))
            epsp = stack.enter_context(tc.tile_pool(name="eps", bufs=4, space="PSUM"))
            tpsp = stack.enter_context(tc.tile_pool(name="tps", bufs=2, space="PSUM"))
            opsp = stack.enter_context(tc.tile_pool(name="ops", bufs=1, space="PSUM"))
            smp = stack.enter_context(tc.tile_pool(name="smp", bufs=2))
            esp = stack.enter_context(tc.tile_pool(name="esp", bufs=6))
            pp = stack.enter_context(tc.tile_pool(name="pp", bufs=2))
            ptp = stack.enter_context(tc.tile_pool(name="ptp", bufs=1))
            obp = stack.enter_context(tc.tile_pool(name="obp", bufs=2))

            gv_t = [None] * NI
            for i in range(4):
                gv_t[i] = qtp.tile([128, NK, 128], FP16, name=f"gv{i}",
                                   tag="gv")
                nc.sync.dma_start(gv_t[i], gt2[i])
            # x^T chunk loads, column-progressive, preps split SP/ACT
            xth_sb = []
            for m in range(NK):
                xth_sb.append(ktp.tile([128, S], FP16, name=f"xth{m}",
                                       tag=f"xth{m}"))
            vv_all = vvp.tile([128, NT, D], DBF, name="vv_all", tag="vv")
            for (c0, c1) in ((0, 512), (512, 1536), (1536, 3072)):
                for m in range(NK):
                    eng = nc.sync if m % 2 == 0 else nc.gpsimd
                    eng.dma_start(xth_sb[m][:, c0:c1],
                                  xth[m * 128:(m + 1) * 128, c0:c1])
            # first V half ahead of the last x^T chunk: P@V(0) needs it
            # before the E stream needs cols 3072:4096
            nc.sync.dma_start(vv_all[:, 0:16, :], vin[:, 0:16, :])
            for m in range(NK):
                eng = nc.sync if m % 2 == 0 else nc.gpsimd
                eng.dma_start(xth_sb[m][:, 3072:4096],
                              xth[m * 128:(m + 1) * 128, 3072:4096])
            nc.sync.dma_start(vv_all[:, 16:NT, :], vin[:, 16:NT, :])

            st_mx8 = [None, None]
            eq_t = [[None, None], [None, None]]   # [i%2][half]

            def e_block(i, jb):
                sl = slice(jb * 512, (jb + 1) * 512)
                ps = epsp.tile([128, 512], FP32, name=f"eps{i}_{jb}", tag="eps")
                for k in range(NK):
                    nc.tensor.matmul(ps, lhsT=gv_t[i][:, k, :],
                                     rhs=xth_sb[k][:, sl],
                                     start=(k == 0), stop=(k == NK - 1))
                half = jb % 4
                nc.scalar.activation(
                    eq_t[i % 2][jb // 4][:, half * 512:(half + 1) * 512], ps,
                    Copy)
                nc.vector.reduce_max(st_mx8[i % 2][:, jb:jb + 1], ps, axis=AX)

            def softmax_issue(i):
                """Global max + exp chain (DVE stats + ACT exps) for i."""
                mx8 = st_mx8[i % 2]
                mrow = smp.tile([128, 1], FP32, name=f"mrow{i}", tag="mrow")
                nc.vector.reduce_max(mrow, mx8, axis=AX)
                negm = smp.tile([128, 1], FP32, name=f"negm{i}", tag="negm")
                nc.vector.tensor_scalar_mul(negm, mrow, -SCALE)
                # P in two half-row tiles so the next i's exp can start as
                # soon as the first half's transposes have consumed it
                p_h = [pp.tile([128, S // 2], DBF, name=f"p{i}_{h}", tag="p")
                       for h in range(2)]
                lp8 = smp.tile([128, NJB], FP32, name=f"lp8_{i}", tag="lp8")
                for jb in range(NJB):
                    half = jb % 4
                    nc.scalar.activation(
                        p_h[jb // 4][:, (jb % 4) * 512:(jb % 4) * 512 + 512],
                        eq_t[i % 2][jb // 4][:, half * 512:(half + 1) * 512],
                        Exp, bias=negm, scale=SCALE,
                        accum_out=lp8[:, jb:jb + 1],
                    )
                lrow = smp.tile([128, 1], FP32, name=f"lrow{i}", tag="lrow")
                nc.vector.reduce_sum(lrow, lp8, axis=AX)
                linv = smp.tile([128, 1], FP32, name=f"linv{i}", tag="linv")
                nc.vector.reciprocal(linv, lrow)
                return p_h, linv

            _uid = [0]
            pt_sb = ptp.tile([128, NT, 128], DBF, name="pt", tag="pt")

            def pt_group(p_h, g):
                """Transpose 4 P tiles (4g..4g+3) via one psum bank group."""
                _uid[0] += 1
                tp = tpsp.tile([128, 512], DBF, name=f"tpg{_uid[0]}", tag="tp")
                for w in range(4):
                    tl = (4 * g + w) % 16
                    nc.tensor.transpose(tp[:, w * 128:(w + 1) * 128],
                                        p_h[g // 4][:, tl * 128:(tl + 1) * 128],
                                        ident)
                nc.vector.tensor_copy(
                    pt_sb[:, 4 * g:4 * g + 4, :].rearrange("p t f -> p (t f)"), tp)

            def pv_sweep(i, p_h, linv):
                """P^T + P@V' + 1/l scale + row store for i-tile i."""
                op0 = opsp.tile([128, 512], FP32, name=f"op0_{i}", tag="op0")
                op1 = opsp.tile([128, 512], FP32, name=f"op1_{i}", tag="op1")
                for g in range(NT // 4):
                    pt_group(p_h, g)
                    for w in range(4):
                        t = 4 * g + w
                        nc.tensor.matmul(op0, lhsT=pt_sb[:, t, :],
                                         rhs=vv_all[:, t, 0:512],
                                         start=(t == 0), stop=(t == NT - 1))
                        nc.tensor.matmul(op1, lhsT=pt_sb[:, t, :],
                                         rhs=vv_all[:, t, 512:D],
                                         start=(t == 0), stop=(t == NT - 1))
                osb = obp.tile([128, D], FP16, name=f"osb{i}", tag="osb")
                nc.vector.tensor_scalar_mul(osb[:, 0:512], op0, linv)
                nc.vector.tensor_scalar_mul(osb[:, 512:D], op1, linv)
                nc.sync.dma_start(y[i * 128:(i + 1) * 128, :], osb)

            sm = [None] * NI

            def E_tile(i, jbs):
                if jbs[0] == 0:
                    st_mx8[i % 2] = smp.tile([128, NJB], FP32, name=f"mx8_{i}",
                                             tag=f"mx8{i % 2}")
                for jb in jbs:
                    if jb % 4 == 0:
                        eq_t[i % 2][jb // 4] = esp.tile(
                            [128, 2048], FP32, name=f"e{i}_{jb // 4}", tag="e")
                    e_block(i, jb)

            # head: E(0)/E(1) interleaved jb-major over the x^T chunk
            # supply; E(0) finishes first so exp(0) starts early
            for jb in range(6):
                E_tile(0, [jb])
                E_tile(1, [jb])
            E_tile(0, [6, 7])
            sm[0] = softmax_issue(0)
            E_tile(1, [6, 7])
            sm[1] = softmax_issue(1)
            # lag-2 pipeline: E(i); sweep(i-2)
            for i in range(2, NI):
                E_tile(i, list(range(NJB)))
                if i + 2 < NI:
                    gv_t[i + 2] = qtp.tile([128, NK, 128], FP16,
                                           name=f"gv{i + 2}", tag="gv")
                    nc.sync.dma_start(gv_t[i + 2], gt2[i + 2])
                sm[i] = softmax_issue(i)
                pv_sweep(i - 2, sm[i - 2][0], sm[i - 2][1])
            pv_sweep(NI - 2, sm[NI - 2][0], sm[NI - 2][1])
            pv_sweep(NI - 1, sm[NI - 1][0], sm[NI - 1][1])
    nc.compile()
    return nc


def _get_programs():
    if "nc1" not in _cache:
        _cache["nc1"] = _build_phase1()
        _cache["nc2"] = _build_phase2()
    return _cache["nc1"], _cache["nc2"]


def kernel(x, Wq, Wk, Wv, Wo):
    from concourse.bass_utils import run_bass_kernel_spmd

    nc1, nc2 = _get_programs()

    x = np.asarray(x, dtype=np.float32)
    # fold the weights once on the host (associativity):
    #   energy = x (Wq Wk^T) x^T ;  out = attn (x (Wv Wo))
    wm = (np.asarray(Wq, np.float64) @ np.asarray(Wk, np.float64).T
          ).astype(np.float32)
    wvo = (np.asarray(Wv, np.float64) @ np.asarray(Wo, np.float64)
           ).astype(np.float32)

    # ---- phase 1: per-core row slices ----
    in1 = []
    for c in range(8):
        b, i = divmod(c, 4)
        rows = x[b, i * BLK:(i + 1) * BLK, :]           # [BLK, D]
        in1.append({
            "xt": np.ascontiguousarray(rows.T),
            "wm": wm, "wvo": wvo,
        })
    res1 = run_bass_kernel_spmd(nc1, in1, list(range(8))).results

    # ---- host gather of V' shards; pre-cast x^T per batch ----
    xth_full, v_full = [], []
    for b in range(B):
        xth_full.append(np.ascontiguousarray(
            x[b].T.astype(np.float16)))                  # [D, S]
        v = np.concatenate(
            [res1[b * 4 + i]["vo"] for i in range(4)], axis=0)    # [NT, 128, D]
        v_full.append(np.ascontiguousarray(v.transpose(1, 0, 2)))  # [128, NT, D]

    # ---- phase 2 ----
    in2 = []
    for c in range(8):
        b, i = divmod(c, 4)
        gstack = res1[c]["gt"].reshape(NK, 128, NI, 128)  # [n, p, i, f]
        in2.append({
            "xth": xth_full[b], "vin": v_full[b],
            "gt2": np.ascontiguousarray(gstack.transpose(2, 1, 0, 3)),
        })
    res2 = run_bass_kernel_spmd(nc2, in2, list(range(8))).results

    out = np.empty((B, S, D), dtype=np.float32)
    for c in range(8):
        b, i = divmod(c, 4)
        out[b, i * BLK:(i + 1) * BLK, :] = res2[c]["y"].astype(np.float32)
    return out


# revision 37
# speedup vs baseline: 2.1580x; 1.0370x over previous
"""Trainium2 Bass kernel for single-head self-attention (B=2, S=4096, D=1024).

reference:
    q = x @ Wq; k = x @ Wk; v = x @ Wv          # [B,S,D]
    energy = einsum('bid,bjd->bij', q, k) * 8.0  # SCALE = sqrt(64)
    attn = softmax(energy, axis=-1)
    out = einsum('bij,bjd->bid', attn, v) @ Wo

Weight folding (associativity): energy = x @ (Wq Wk^T) @ x^T and
out = attn @ (x @ (Wv Wo)), so the host precomputes M = Wq @ Wk^T and
W' = Wv @ Wo once (fp64) and the device only runs TWO projections
(G = x@M, V' = x@W') plus the two S^2-sized attention matmuls -- the
separate K projection and the output projection disappear.

Two SPMD launches over 8 cores (= 2 batches x 4 query-blocks of 1024):
  phase 1: each core computes G / V' for its own 1024 rows; the host
           gathers V' shards (and pre-casts x^T to fp16) per batch.
  phase 2: each core computes softmax(G_blk @ x^T * 8) @ V' for its
           1024 queries against the full batch; output rows come out
           of the P@V' accumulation directly.

Precision: logits have std ~256 (SCALE multiplies), so the logit path
needs much better than bf16 input precision.  The G projection runs in
the PE's fp32r mode (single-pass at the bf16 rate for free-dim >= 256,
~13 effective mantissa bits); G and x^T are stored as fp16 (2^-11
rounding) and G @ x^T runs in fp16 (fp16 products are exact in fp32
PSUM accumulation).  Combined logit noise ~0.11 std -> ~6e-3 output
rel err.  V' / P run in bf16.

Phase-2 layout: x^T (fp16, 64KB/part) and V' (bf16, 64KB/part) are
SBUF-resident, so after the initial load the attention sweep runs with
no input DMA.  A dma_start occupies its issuing queue through the
whole transfer, so queue assignment is part of the schedule: bulk
loads ride SP in consumption order, the ACT queue stays clear for the
softmax exp chain, and E psum->SBUF copies + P^T copies run on Pool.
"""

import numpy as np
import ml_dtypes

B, S, D = 2, 4096, 1024
BLK = 1024          # queries per core
SCALE = 8.0         # HEAD_DIM ** 0.5 = sqrt(64)
NK = D // 128       # 8 k-tiles over the feature dim
NT = S // 128       # 32 j-tiles over keys
NI = BLK // 128     # 8 i-tiles over this core's queries
NJB = S // 512      # 8 key blocks of 512
BF16 = ml_dtypes.bfloat16

_cache = {}


def _build_phase1():
    """G = x@M and V' = x@W' for this core's 1024 rows (fp32r single-pass)."""
    import concourse.mybir as mybir
    from concourse import bacc
    from concourse.tile import TileContext

    FP16 = mybir.dt.float16
    FP32 = mybir.dt.float32
    FP32R = mybir.dt.float32r
    DBF = mybir.dt.bfloat16

    nc = bacc.Bacc("TRN2", target_bir_lowering=False, debug=False, num_devices=8)

    xt = nc.dram_tensor("xt", [D, BLK], FP32R, kind="ExternalInput")  # rows.T
    wm = nc.dram_tensor("wm", [D, D], FP32R, kind="ExternalInput")    # Wq@Wk^T
    wvo = nc.dram_tensor("wvo", [D, D], FP32R, kind="ExternalInput")  # Wv@Wo
    gt = nc.dram_tensor("gt", [D, BLK], FP16, kind="ExternalOutput")
    vo = nc.dram_tensor("vo", [NI, 128, D], DBF, kind="ExternalOutput")

    with TileContext(nc) as tc:
      with (
          tc.tile_pool(name="xp", bufs=1) as xp,
          tc.tile_pool(name="wp", bufs=1) as wp,
          tc.tile_pool(name="gps", bufs=4, space="PSUM") as gps,
          tc.tile_pool(name="gst", bufs=1) as gstp,
          tc.tile_pool(name="vps", bufs=2, space="PSUM") as vps,
          tc.tile_pool(name="vsb", bufs=3) as vsbp,
      ):
        # all loads on the SP queue in consumption order: the shared DMA
        # bus serializes transfers, so a single queue in priority order
        # beats spreading (racing queues invert priorities)
        xt_r = xt[:, :].rearrange("(n p) s -> p n s", p=128)
        x_sb = xp.tile([128, NK, BLK], FP32R, name="x_sb", tag="x_sb")
        wm_sb = wp.tile([128, NK, D], FP32R, name="wm_sb", tag="wm_sb")
        wm_r = wm[:, :].rearrange("(n p) d -> p n d", p=128)
        nc.sync.dma_start(x_sb[:, :, 0:256], xt_r[:, :, 0:256])
        nc.sync.dma_start(wm_sb[:, :, 0:256], wm_r[:, :, 0:256])
        nc.sync.dma_start(x_sb[:, :, 256:512], xt_r[:, :, 256:512])
        nc.sync.dma_start(wm_sb[:, :, 256:512], wm_r[:, :, 256:512])
        nc.sync.dma_start(x_sb[:, :, 512:BLK], xt_r[:, :, 512:BLK])
        nc.sync.dma_start(wm_sb[:, :, 512:768], wm_r[:, :, 512:768])
        nc.sync.dma_start(wm_sb[:, :, 768:D], wm_r[:, :, 768:D])
        wv_sb = wp.tile([128, NK, D], FP32R, name="wv_sb", tag="wv_sb")
        wvo_r = wvo[:, :].rearrange("(n p) d -> p n d", p=128)
        nc.sync.dma_start(wv_sb[:, :, 0:512], wvo_r[:, :, 0:512])
        nc.sync.dma_start(wv_sb[:, :, 512:D], wvo_r[:, :, 512:D])

        # G blocks emitted in DMA-supply order: each group becomes runnable
        # as one more of the loads above lands.
        K_ORDER = [
            (0, 0), (0, 1),
            (1, 0), (1, 1),
            (0, 2), (0, 3), (1, 2), (1, 3),
            (2, 0), (2, 1), (2, 2), (2, 3),
            (0, 4), (0, 5), (1, 4), (1, 5), (2, 4), (2, 5),
            (0, 6), (0, 7), (1, 6), (1, 7), (2, 6), (2, 7),
        ]
        NBS = ((0, 256), (256, 256), (512, 512))
        st = []
        for m in range(NK):
            st.append(gstp.tile([128, BLK], FP16, name=f"gs{m}", tag=f"s{m}"))
        for (nb, m) in K_ORDER:
            n0, nw = NBS[nb]
            nsl = slice(n0, n0 + nw)
            msl = slice(m * 128, (m + 1) * 128)
            ps = gps.tile([128, 512], FP32, name=f"gp{n0}_{m}", tag="ps")
            for k in range(NK):
                nc.tensor.matmul(ps[:, 0:nw], lhsT=wm_sb[:, k, msl],
                                 rhs=x_sb[:, k, nsl],
                                 start=(k == 0), stop=(k == NK - 1))
            nc.vector.tensor_copy(st[m][:, nsl], ps[:, 0:nw])
        for m in range(NK):
            nc.sync.dma_start(gt[m * 128:(m + 1) * 128, :], st[m])

        for j in range(NI):
            jsl = slice(j * 128, (j + 1) * 128)
            vt = vsbp.tile([128, D], DBF, name=f"vt{j}", tag="vt")
            for db in range(2):
                ps = vps.tile([128, 512], FP32, name=f"vps{j}_{db}", tag="vps")
                for k in range(NK):
                    nc.tensor.matmul(
                        ps, lhsT=x_sb[:, k, jsl],
                        rhs=wv_sb[:, k, db * 512:(db + 1) * 512],
                        start=(k == 0), stop=(k == NK - 1),
                    )
                nc.vector.tensor_copy(vt[:, db * 512:(db + 1) * 512], ps)
                nc.scalar.dma_start(vo[j][:, db * 512:(db + 1) * 512],
                                    vt[:, db * 512:(db + 1) * 512])
    nc.compile()
    return nc


def _build_phase2():
    """softmax(G_blk @ x^T * 8) @ V' for this core's 1024 queries."""
    import concourse.mybir as mybir
    from concourse import bacc
    from concourse.tile import TileContext
    from concourse.masks import make_identity

    FP16 = mybir.dt.float16
    FP32 = mybir.dt.float32
    DBF = mybir.dt.bfloat16
    Exp = mybir.ActivationFunctionType.Exp
    Copy = mybir.ActivationFunctionType.Copy
    AX = mybir.AxisListType.X

    nc = bacc.Bacc("TRN2", target_bir_lowering=False, debug=False, num_devices=8)

    xth = nc.dram_tensor("xth", [D, S], FP16, kind="ExternalInput")
    # per-i-tile partition-major G: [i, p, n, f] = gt[n*128+p, i*128+f]
    gt2 = nc.dram_tensor("gt2", [NI, 128, NK, 128], FP16, kind="ExternalInput")
    # partition-major V': [p, t, d] = V'[t*128+p, d]
    vin = nc.dram_tensor("vin", [128, NT, D], DBF, kind="ExternalInput")
    y = nc.dram_tensor("y", [BLK, D], FP16, kind="ExternalOutput")

    from contextlib import ExitStack
    with TileContext(nc) as tc:
        with ExitStack() as stack:
            constp = stack.enter_context(tc.tile_pool(name="const", bufs=1))
            ident = constp.tile([128, 128], DBF)
            make_identity(nc, ident)

            ktp = stack.enter_context(tc.tile_pool(name="ktp", bufs=1))
            qtp = stack.enter_context(tc.tile_pool(name="qtp", bufs=4))
            vvp = stack.enter_context(tc.tile_pool(name="vvp", bufs=1))
            epsp = stack.enter_context(tc.tile_pool(name="eps", bufs=4, space="PSUM"))
            tpsp = stack.enter_context(tc.tile_pool(name="tps", bufs=2, space="PSUM"))
            opsp = stack.enter_context(tc.tile_pool(name="ops", bufs=1, space="PSUM"))
            smp = stack.enter_context(tc.tile_pool(name="smp", bufs=2))
            esp = stack.enter_context(tc.tile_pool(name="esp", bufs=5))
            pp = stack.enter_context(tc.tile_pool(name="pp", bufs=2))
            ptp = stack.enter_context(tc.tile_pool(name="ptp", bufs=1))
            obp = stack.enter_context(tc.tile_pool(name="obp", bufs=2))

            gv_t = [None] * NI

            def gv(i):
                return gv_t[i]

            gv_t[0] = qtp.tile([128, NK, 128], FP16, name="gv0", tag="gv")
            nc.sync.dma_start(gv_t[0], gt2[0])
            # x^T chunk loads, column-progressive.  SP carries the even m
            # rows in 4 chunks; the Pool SWDGE queue (994ns fixed cost per
            # descriptor-gen) carries the odd rows in 3 bigger chunks.
            xth_sb = []
            for m in range(NK):
                xth_sb.append(ktp.tile([128, S], FP16, name=f"xth{m}",
                                       tag=f"xth{m}"))
            vv_all = vvp.tile([128, NT, D], DBF, name="vv_all", tag="vv")
            for m in range(0, NK, 2):
                nc.sync.dma_start(xth_sb[m][:, 0:512],
                                  xth[m * 128:(m + 1) * 128, 0:512])
            for m in range(1, NK, 2):
                nc.gpsimd.dma_start(xth_sb[m][:, 0:512],
                                    xth[m * 128:(m + 1) * 128, 0:512])
            for i in range(1, 4):
                gv_t[i] = qtp.tile([128, NK, 128], FP16, name=f"gv{i}",
                                   tag="gv")
                nc.sync.dma_start(gv_t[i], gt2[i])
            for m in range(1, NK, 2):
                nc.gpsimd.dma_start(xth_sb[m][:, 512:3072],
                                    xth[m * 128:(m + 1) * 128, 512:3072])
            for (c0, c1) in ((512, 1536), (1536, 3072)):
                for m in range(0, NK, 2):
                    nc.sync.dma_start(xth_sb[m][:, c0:c1],
                                      xth[m * 128:(m + 1) * 128, c0:c1])
            nc.sync.dma_start(vv_all[:, 0:16, :], vin[:, 0:16, :])
            for m in range(NK):
                eng = nc.sync if m % 2 == 0 else nc.gpsimd
                eng.dma_start(xth_sb[m][:, 3072:4096],
                              xth[m * 128:(m + 1) * 128, 3072:4096])
            nc.sync.dma_start(vv_all[:, 16:NT, :], vin[:, 16:NT, :])

            st_mx8 = [None, None]
            eq_t = [[None, None], [None, None]]   # [i%2][half]

            def e_block(i, jb):
                sl = slice(jb * 512, (jb + 1) * 512)
                ps = epsp.tile([128, 512], FP32, name=f"eps{i}_{jb}", tag="eps")
                for k in range(NK):
                    nc.tensor.matmul(ps, lhsT=gv(i)[:, k, :],
                                     rhs=xth_sb[k][:, sl],
                                     start=(k == 0), stop=(k == NK - 1))
                half = jb % 4
                nc.scalar.activation(
                    eq_t[i % 2][jb // 4][:, half * 512:(half + 1) * 512], ps,
                    Copy)
                nc.vector.reduce_max(st_mx8[i % 2][:, jb:jb + 1], ps, axis=AX)

            def softmax_issue(i):
                """Global max + exp chain (DVE stats + ACT exps) for i."""
                mx8 = st_mx8[i % 2]
                mrow = smp.tile([128, 1], FP32, name=f"mrow{i}", tag="mrow")
                nc.vector.reduce_max(mrow, mx8, axis=AX)
                negm = smp.tile([128, 1], FP32, name=f"negm{i}", tag="negm")
                nc.vector.tensor_scalar_mul(negm, mrow, -SCALE)
                # P in two half-row tiles so the next i's exp can start as
                # soon as the first half's transposes have consumed it
                p_h = [pp.tile([128, S // 2], DBF, name=f"p{i}_{h}", tag="p")
                       for h in range(2)]
                lp8 = smp.tile([128, NJB], FP32, name=f"lp8_{i}", tag="lp8")
                for jb in range(NJB):
                    half = jb % 4
                    nc.scalar.activation(
                        p_h[jb // 4][:, (jb % 4) * 512:(jb % 4) * 512 + 512],
                        eq_t[i % 2][jb // 4][:, half * 512:(half + 1) * 512],
                        Exp, bias=negm, scale=SCALE,
                        accum_out=lp8[:, jb:jb + 1],
                    )
                lrow = smp.tile([128, 1], FP32, name=f"lrow{i}", tag="lrow")
                nc.vector.reduce_sum(lrow, lp8, axis=AX)
                linv = smp.tile([128, 1], FP32, name=f"linv{i}", tag="linv")
                nc.vector.reciprocal(linv, lrow)
                return p_h, linv

            _uid = [0]
            pt_sb = ptp.tile([128, NT, 128], DBF, name="pt", tag="pt")

            def pt_group(p_h, g):
                """Transpose 4 P tiles (4g..4g+3) via one psum bank group."""
                _uid[0] += 1
                tp = tpsp.tile([128, 512], DBF, name=f"tpg{_uid[0]}", tag="tp")
                for w in range(4):
                    tl = (4 * g + w) % 16
                    nc.tensor.transpose(tp[:, w * 128:(w + 1) * 128],
                                        p_h[g // 4][:, tl * 128:(tl + 1) * 128],
                                        ident)
                nc.vector.tensor_copy(
                    pt_sb[:, 4 * g:4 * g + 4, :].rearrange("p t f -> p (t f)"), tp)

            def pv_sweep(i, p_h, linv, last=False):
                """P^T + P@V' + 1/l scale + row store for i-tile i.

                P^T groups are issued two ahead of their consuming matmuls
                so the psum->pt copy latency hides under PE work.  For the
                last i-tile the two output halves run as separate t-sweeps
                so half 0 stores while half 1 computes (shorter drain).
                """
                op0 = opsp.tile([128, 512], FP32, name=f"op0_{i}", tag="op0")
                op1 = opsp.tile([128, 512], FP32, name=f"op1_{i}", tag="op1")
                osb = obp.tile([128, D], FP16, name=f"osb{i}", tag="osb")
                halves = ((op0, 0, 512), (op1, 512, D)) if last else None
                if last:
                    pt_group(p_h, 0)
                    pt_group(p_h, 1)
                    for (op, d0, d1) in halves:
                        for t in range(NT):
                            g = t // 4
                            if t % 4 == 0 and g + 2 < NT // 4 and op is op0:
                                pt_group(p_h, g + 2)
                            nc.tensor.matmul(op, lhsT=pt_sb[:, t, :],
                                             rhs=vv_all[:, t, d0:d1],
                                             start=(t == 0), stop=(t == NT - 1))
                        nc.vector.tensor_scalar_mul(osb[:, d0:d1], op, linv)
                        nc.sync.dma_start(y[i * 128:(i + 1) * 128, d0:d1],
                                          osb[:, d0:d1])
                    return
                pt_group(p_h, 0)
                pt_group(p_h, 1)
                for g in range(NT // 4):
                    if g + 2 < NT // 4:
                        pt_group(p_h, g + 2)
                    for w in range(4):
                        t = 4 * g + w
                        nc.tensor.matmul(op0, lhsT=pt_sb[:, t, :],
                                         rhs=vv_all[:, t, 0:512],
                                         start=(t == 0), stop=(t == NT - 1))
                        nc.tensor.matmul(op1, lhsT=pt_sb[:, t, :],
                                         rhs=vv_all[:, t, 512:D],
                                         start=(t == 0), stop=(t == NT - 1))
                nc.vector.tensor_scalar_mul(osb[:, 0:512], op0, linv)
                nc.vector.tensor_scalar_mul(osb[:, 512:D], op1, linv)
                nc.sync.dma_start(y[i * 128:(i + 1) * 128, :], osb)

            sm = [None] * NI

            def E_tile(i, jbs):
                if jbs[0] == 0:
                    st_mx8[i % 2] = smp.tile([128, NJB], FP32, name=f"mx8_{i}",
                                             tag=f"mx8{i % 2}")
                for jb in jbs:
                    if jb % 4 == 0:
                        eq_t[i % 2][jb // 4] = esp.tile(
                            [128, 2048], FP32, name=f"e{i}_{jb // 4}", tag="e")
                    e_block(i, jb)

            # head: E(0)/E(1) interleaved jb-major over the x^T chunk
            # supply; E(0) finishes first so exp(0) starts early
            for jb in range(6):
                E_tile(0, [jb])
                E_tile(1, [jb])
            E_tile(0, [6, 7])
            sm[0] = softmax_issue(0)
            E_tile(1, [6, 7])
            # lag-2 pipeline: E(i); exps(i-1) (after E(i)'s psum copies so
            # the ACT queue drains in dependency order); sweep(i-2)
            for i in range(2, NI):
                E_tile(i, list(range(NJB)))
                if i + 2 < NI:
                    gv_t[i + 2] = qtp.tile([128, NK, 128], FP16,
                                           name=f"gv{i + 2}", tag="gv")
                    nc.sync.dma_start(gv_t[i + 2], gt2[i + 2])
                sm[i - 1] = softmax_issue(i - 1)
                pv_sweep(i - 2, sm[i - 2][0], sm[i - 2][1])
            sm[NI - 1] = softmax_issue(NI - 1)
            pv_sweep(NI - 2, sm[NI - 2][0], sm[NI - 2][1])
            pv_sweep(NI - 1, sm[NI - 1][0], sm[NI - 1][1], last=True)
    nc.compile()
    return nc


def _get_programs():
    if "nc1" not in _cache:
        _cache["nc1"] = _build_phase1()
        _cache["nc2"] = _build_phase2()
    return _cache["nc1"], _cache["nc2"]


def kernel(x, Wq, Wk, Wv, Wo):
    from concourse.bass_utils import run_bass_kernel_spmd

    nc1, nc2 = _get_programs()

    x = np.asarray(x, dtype=np.float32)
    # fold the weights once on the host (associativity):
    #   energy = x (Wq Wk^T) x^T ;  out = attn (x (Wv Wo))
    wm = (np.asarray(Wq, np.float64) @ np.asarray(Wk, np.float64).T
          ).astype(np.float32)
    wvo = (np.asarray(Wv, np.float64) @ np.asarray(Wo, np.float64)
           ).astype(np.float32)

    # ---- phase 1: per-core row slices ----
    in1 = []
    for c in range(8):
        b, i = divmod(c, 4)
        rows = x[b, i * BLK:(i + 1) * BLK, :]           # [BLK, D]
        in1.append({
            "xt": np.ascontiguousarray(rows.T),
            "wm": wm, "wvo": wvo,
        })
    res1 = run_bass_kernel_spmd(nc1, in1, list(range(8))).results

    # ---- host gather of V' shards; pre-cast x^T per batch ----
    xth_full, v_full = [], []
    for b in range(B):
        xth_full.append(np.ascontiguousarray(
            x[b].T.astype(np.float16)))                  # [D, S]
        v = np.concatenate(
            [res1[b * 4 + i]["vo"] for i in range(4)], axis=0)    # [NT, 128, D]
        v_full.append(np.ascontiguousarray(v.transpose(1, 0, 2)))  # [128, NT, D]

    # ---- phase 2 ----
    in2 = []
    for c in range(8):
        b, i = divmod(c, 4)
        gstack = res1[c]["gt"].reshape(NK, 128, NI, 128)  # [n, p, i, f]
        in2.append({
            "xth": xth_full[b], "vin": v_full[b],
            "gt2": np.ascontiguousarray(gstack.transpose(2, 1, 0, 3)),
        })
    res2 = run_bass_kernel_spmd(nc2, in2, list(range(8))).results

    out = np.empty((B, S, D), dtype=np.float32)
    for c in range(8):
        b, i = divmod(c, 4)
        out[b, i * BLK:(i + 1) * BLK, :] = res2[c]["y"].astype(np.float32)
    return out


# revision 42
# speedup vs baseline: 2.2268x; 1.0319x over previous
"""Trainium2 Bass kernel for single-head self-attention (B=2, S=4096, D=1024).

reference:
    q = x @ Wq; k = x @ Wk; v = x @ Wv          # [B,S,D]
    energy = einsum('bid,bjd->bij', q, k) * 8.0  # SCALE = sqrt(64)
    attn = softmax(energy, axis=-1)
    out = einsum('bij,bjd->bid', attn, v) @ Wo

Weight folding (associativity): energy = x @ (Wq Wk^T) @ x^T and
out = attn @ (x @ (Wv Wo)), so the host precomputes M = Wq @ Wk^T and
W' = Wv @ Wo once (fp64) and the device only runs TWO projections
(G = x@M, V' = x@W') plus the two S^2-sized attention matmuls -- the
separate K projection and the output projection disappear.

Two SPMD launches over 8 cores (= 2 batches x 4 query-blocks of 1024):
  phase 1: each core computes G / V' for its own 1024 rows; the host
           gathers V' shards (and pre-casts x^T to fp16) per batch.
  phase 2: each core computes softmax(G_blk @ x^T * 8) @ V' for its
           1024 queries against the full batch; output rows come out
           of the P@V' accumulation directly.

Precision: logits have std ~256 (SCALE multiplies), so the logit path
needs much better than bf16 input precision.  The G projection runs in
the PE's fp32r mode (single-pass at the bf16 rate for free-dim >= 256,
~13 effective mantissa bits); G and x^T are stored as fp16 (2^-11
rounding) and G @ x^T runs in fp16 (fp16 products are exact in fp32
PSUM accumulation).  Combined logit noise ~0.11 std -> ~6e-3 output
rel err.  V' / P run in bf16.

Phase-2 layout: x^T (fp16, 64KB/part) and V' (bf16, 64KB/part) are
SBUF-resident, so after the initial load the attention sweep runs with
no input DMA.  A dma_start occupies its issuing queue through the
whole transfer, so queue assignment is part of the schedule: bulk
loads ride SP in consumption order, the ACT queue stays clear for the
softmax exp chain, and E psum->SBUF copies + P^T copies run on Pool.
"""

import numpy as np
import ml_dtypes

B, S, D = 2, 4096, 1024
BLK = 1024          # queries per core
SCALE = 8.0         # HEAD_DIM ** 0.5 = sqrt(64)
NK = D // 128       # 8 k-tiles over the feature dim
NT = S // 128       # 32 j-tiles over keys
NI = BLK // 128     # 8 i-tiles over this core's queries
NJB = S // 512      # 8 key blocks of 512
BF16 = ml_dtypes.bfloat16

_cache = {}


def _build_phase1():
    """G = x@M and V' = x@W' for this core's 1024 rows (fp32r single-pass)."""
    import concourse.mybir as mybir
    from concourse import bacc
    from concourse.tile import TileContext

    FP16 = mybir.dt.float16
    FP32 = mybir.dt.float32
    FP32R = mybir.dt.float32r
    DBF = mybir.dt.bfloat16

    nc = bacc.Bacc("TRN2", target_bir_lowering=False, debug=False, num_devices=8)

    xt = nc.dram_tensor("xt", [D, BLK], FP32R, kind="ExternalInput")  # rows.T
    wm = nc.dram_tensor("wm", [D, D], FP32R, kind="ExternalInput")    # Wq@Wk^T
    wvo = nc.dram_tensor("wvo", [D, D], FP32R, kind="ExternalInput")  # Wv@Wo
    gt = nc.dram_tensor("gt", [D, BLK], FP16, kind="ExternalOutput")
    vo = nc.dram_tensor("vo", [NI, 128, D], DBF, kind="ExternalOutput")

    with TileContext(nc) as tc:
      with (
          tc.tile_pool(name="xp", bufs=1) as xp,
          tc.tile_pool(name="wp", bufs=1) as wp,
          tc.tile_pool(name="gps", bufs=4, space="PSUM") as gps,
          tc.tile_pool(name="gst", bufs=1) as gstp,
          tc.tile_pool(name="vps", bufs=2, space="PSUM") as vps,
          tc.tile_pool(name="vsb", bufs=3) as vsbp,
      ):
        # all loads on the SP queue in consumption order: the shared DMA
        # bus serializes transfers, so a single queue in priority order
        # beats spreading (racing queues invert priorities)
        xt_r = xt[:, :].rearrange("(n p) s -> p n s", p=128)
        x_sb = xp.tile([128, NK, BLK], FP32R, name="x_sb", tag="x_sb")
        wm_sb = wp.tile([128, NK, D], FP32R, name="wm_sb", tag="wm_sb")
        wm_r = wm[:, :].rearrange("(n p) d -> p n d", p=128)
        nc.sync.dma_start(x_sb[:, :, 0:256], xt_r[:, :, 0:256])
        nc.sync.dma_start(wm_sb[:, :, 0:256], wm_r[:, :, 0:256])
        nc.sync.dma_start(x_sb[:, :, 256:512], xt_r[:, :, 256:512])
        nc.sync.dma_start(wm_sb[:, :, 256:512], wm_r[:, :, 256:512])
        nc.sync.dma_start(x_sb[:, :, 512:BLK], xt_r[:, :, 512:BLK])
        nc.sync.dma_start(wm_sb[:, :, 512:768], wm_r[:, :, 512:768])
        nc.sync.dma_start(wm_sb[:, :, 768:D], wm_r[:, :, 768:D])
        wv_sb = wp.tile([128, NK, D], FP32R, name="wv_sb", tag="wv_sb")
        wvo_r = wvo[:, :].rearrange("(n p) d -> p n d", p=128)
        nc.sync.dma_start(wv_sb[:, :, 0:512], wvo_r[:, :, 0:512])
        nc.sync.dma_start(wv_sb[:, :, 512:D], wvo_r[:, :, 512:D])

        # G blocks emitted in DMA-supply order: each group becomes runnable
        # as one more of the loads above lands.
        K_ORDER = [
            (0, 0), (0, 1),
            (1, 0), (1, 1),
            (0, 2), (0, 3), (1, 2), (1, 3),
            (2, 0), (2, 1), (2, 2), (2, 3),
            (0, 4), (0, 5), (1, 4), (1, 5), (2, 4), (2, 5),
            (0, 6), (0, 7), (1, 6), (1, 7), (2, 6), (2, 7),
        ]
        NBS = ((0, 256), (256, 256), (512, 512))
        st = []
        for m in range(NK):
            st.append(gstp.tile([128, BLK], FP16, name=f"gs{m}", tag=f"s{m}"))
        for (nb, m) in K_ORDER:
            n0, nw = NBS[nb]
            nsl = slice(n0, n0 + nw)
            msl = slice(m * 128, (m + 1) * 128)
            ps = gps.tile([128, 512], FP32, name=f"gp{n0}_{m}", tag="ps")
            for k in range(NK):
                nc.tensor.matmul(ps[:, 0:nw], lhsT=wm_sb[:, k, msl],
                                 rhs=x_sb[:, k, nsl],
                                 start=(k == 0), stop=(k == NK - 1))
            nc.vector.tensor_copy(st[m][:, nsl], ps[:, 0:nw])
        for m in range(NK):
            nc.sync.dma_start(gt[m * 128:(m + 1) * 128, :], st[m])

        for j in range(NI):
            jsl = slice(j * 128, (j + 1) * 128)
            vt = vsbp.tile([128, D], DBF, name=f"vt{j}", tag="vt")
            for db in range(2):
                ps = vps.tile([128, 512], FP32, name=f"vps{j}_{db}", tag="vps")
                for k in range(NK):
                    nc.tensor.matmul(
                        ps, lhsT=x_sb[:, k, jsl],
                        rhs=wv_sb[:, k, db * 512:(db + 1) * 512],
                        start=(k == 0), stop=(k == NK - 1),
                    )
                nc.vector.tensor_copy(vt[:, db * 512:(db + 1) * 512], ps)
                nc.scalar.dma_start(vo[j][:, db * 512:(db + 1) * 512],
                                    vt[:, db * 512:(db + 1) * 512])
    nc.compile()
    return nc


def _build_phase2():
    """softmax(G_blk @ x^T * 8) @ V' for this core's 1024 queries."""
    import concourse.mybir as mybir
    from concourse import bacc
    from concourse.tile import TileContext
    from concourse.masks import make_identity

    FP16 = mybir.dt.float16
    FP32 = mybir.dt.float32
    DBF = mybir.dt.bfloat16
    Exp = mybir.ActivationFunctionType.Exp
    Copy = mybir.ActivationFunctionType.Copy
    AX = mybir.AxisListType.X

    nc = bacc.Bacc("TRN2", target_bir_lowering=False, debug=False, num_devices=8)

    xth = nc.dram_tensor("xth", [D, S], FP16, kind="ExternalInput")
    # per-i-tile partition-major G: [i, p, n, f] = gt[n*128+p, i*128+f]
    gt2 = nc.dram_tensor("gt2", [NI, 128, NK, 128], FP16, kind="ExternalInput")
    # partition-major V': [p, t, d] = V'[t*128+p, d]
    vin = nc.dram_tensor("vin", [128, NT, D], DBF, kind="ExternalInput")
    y = nc.dram_tensor("y", [BLK, D], FP16, kind="ExternalOutput")

    from contextlib import ExitStack
    with TileContext(nc) as tc:
        with ExitStack() as stack:
            constp = stack.enter_context(tc.tile_pool(name="const", bufs=1))
            ident = constp.tile([128, 128], DBF)
            make_identity(nc, ident)

            ktp = stack.enter_context(tc.tile_pool(name="ktp", bufs=1))
            qtp = stack.enter_context(tc.tile_pool(name="qtp", bufs=4))
            vvp = stack.enter_context(tc.tile_pool(name="vvp", bufs=1))
            epsp = stack.enter_context(tc.tile_pool(name="eps", bufs=2, space="PSUM"))
            tpsp = stack.enter_context(tc.tile_pool(name="tps", bufs=2, space="PSUM"))
            opsp = stack.enter_context(tc.tile_pool(name="ops", bufs=2, space="PSUM"))
            smp = stack.enter_context(tc.tile_pool(name="smp", bufs=2))
            esp = stack.enter_context(tc.tile_pool(name="esp", bufs=5))
            pp = stack.enter_context(tc.tile_pool(name="pp", bufs=2))
            ptp = stack.enter_context(tc.tile_pool(name="ptp", bufs=1))
            obp = stack.enter_context(tc.tile_pool(name="obp", bufs=2))

            gv_t = [None] * NI

            def gv(i):
                return gv_t[i]

            gv_t[0] = qtp.tile([128, NK, 128], FP16, name="gv0", tag="gv")
            nc.sync.dma_start(gv_t[0], gt2[0])
            # x^T as one [128, NK, S] tile: each column chunk is a single
            # batched DMA covering all 8 k-rows (full bus bandwidth, one
            # queue slot); everything rides SP in consumption order
            xth_r = xth[:, :].rearrange("(n p) s -> p n s", p=128)
            xth_all = ktp.tile([128, NK, S], FP16, name="xth_all", tag="xth")
            xth_sb = [xth_all[:, m, :] for m in range(NK)]
            vv_all = vvp.tile([128, NT, D], DBF, name="vv_all", tag="vv")
            nc.sync.dma_start(xth_all[:, :, 0:512], xth_r[:, :, 0:512])
            for i in range(1, 4):
                gv_t[i] = qtp.tile([128, NK, 128], FP16, name=f"gv{i}",
                                   tag="gv")
                nc.sync.dma_start(gv_t[i], gt2[i])
            nc.sync.dma_start(xth_all[:, :, 512:1536], xth_r[:, :, 512:1536])
            nc.sync.dma_start(xth_all[:, :, 1536:3072], xth_r[:, :, 1536:3072])
            nc.sync.dma_start(xth_all[:, :, 3072:4096], xth_r[:, :, 3072:4096])
            nc.sync.dma_start(vv_all[:, 0:16, :], vin[:, 0:16, :])
            nc.sync.dma_start(vv_all[:, 16:NT, :], vin[:, 16:NT, :])

            st_mx8 = [None, None]
            eq_t = [[None, None], [None, None]]   # [i%2][half]

            def e_block(i, jb):
                sl = slice(jb * 512, (jb + 1) * 512)
                ps = epsp.tile([128, 512], FP32, name=f"eps{i}_{jb}", tag="eps")
                for k in range(NK):
                    nc.tensor.matmul(ps, lhsT=gv(i)[:, k, :],
                                     rhs=xth_sb[k][:, sl],
                                     start=(k == 0), stop=(k == NK - 1))
                half = jb % 4
                nc.scalar.activation(
                    eq_t[i % 2][jb // 4][:, half * 512:(half + 1) * 512], ps,
                    Copy)
                nc.vector.reduce_max(st_mx8[i % 2][:, jb:jb + 1], ps, axis=AX)

            def softmax_issue(i):
                """Global max + exp chain (DVE stats + ACT exps) for i."""
                mx8 = st_mx8[i % 2]
                mrow = smp.tile([128, 1], FP32, name=f"mrow{i}", tag="mrow")
                nc.vector.reduce_max(mrow, mx8, axis=AX)
                negm = smp.tile([128, 1], FP32, name=f"negm{i}", tag="negm")
                nc.vector.tensor_scalar_mul(negm, mrow, -SCALE)
                # P in two half-row tiles so the next i's exp can start as
                # soon as the first half's transposes have consumed it
                p_h = [pp.tile([128, S // 2], DBF, name=f"p{i}_{h}", tag="p")
                       for h in range(2)]
                lp8 = smp.tile([128, NJB], FP32, name=f"lp8_{i}", tag="lp8")
                for jb in range(NJB):
                    half = jb % 4
                    nc.scalar.activation(
                        p_h[jb // 4][:, (jb % 4) * 512:(jb % 4) * 512 + 512],
                        eq_t[i % 2][jb // 4][:, half * 512:(half + 1) * 512],
                        Exp, bias=negm, scale=SCALE,
                        accum_out=lp8[:, jb:jb + 1],
                    )
                lrow = smp.tile([128, 1], FP32, name=f"lrow{i}", tag="lrow")
                nc.vector.reduce_sum(lrow, lp8, axis=AX)
                linv = smp.tile([128, 1], FP32, name=f"linv{i}", tag="linv")
                nc.vector.reciprocal(linv, lrow)
                return p_h, linv

            _uid = [0]
            pt_sb = ptp.tile([128, NT, 128], DBF, name="pt", tag="pt")

            def pt_group(p_h, g):
                """Transpose 4 P tiles (4g..4g+3) via one psum bank group."""
                _uid[0] += 1
                tp = tpsp.tile([128, 512], DBF, name=f"tpg{_uid[0]}", tag="tp")
                for w in range(4):
                    tl = (4 * g + w) % 16
                    nc.tensor.transpose(tp[:, w * 128:(w + 1) * 128],
                                        p_h[g // 4][:, tl * 128:(tl + 1) * 128],
                                        ident)
                nc.vector.tensor_copy(
                    pt_sb[:, 4 * g:4 * g + 4, :].rearrange("p t f -> p (t f)"), tp)

            def pv_sweep(i, p_h, linv, last=False):
                """P^T + P@V' + 1/l scale + row store for i-tile i.

                P^T groups are issued two ahead of their consuming matmuls
                so the psum->pt copy latency hides under PE work.  For the
                last i-tile the two output halves run as separate t-sweeps
                so half 0 stores while half 1 computes (shorter drain).
                """
                op0 = opsp.tile([128, 512], FP32, name=f"op0_{i}", tag="op0")
                op1 = opsp.tile([128, 512], FP32, name=f"op1_{i}", tag="op1")
                osb = obp.tile([128, D], FP16, name=f"osb{i}", tag="osb")
                halves = ((op0, 0, 512), (op1, 512, D)) if last else None
                if last:
                    pt_group(p_h, 0)
                    pt_group(p_h, 1)
                    for (op, d0, d1) in halves:
                        for t in range(NT):
                            g = t // 4
                            if t % 4 == 0 and g + 2 < NT // 4 and op is op0:
                                pt_group(p_h, g + 2)
                            nc.tensor.matmul(op, lhsT=pt_sb[:, t, :],
                                             rhs=vv_all[:, t, d0:d1],
                                             start=(t == 0), stop=(t == NT - 1))
                        nc.vector.tensor_scalar_mul(osb[:, d0:d1], op, linv)
                        nc.sync.dma_start(y[i * 128:(i + 1) * 128, d0:d1],
                                          osb[:, d0:d1])
                    return
                pt_group(p_h, 0)
                pt_group(p_h, 1)
                for g in range(NT // 4):
                    if g + 2 < NT // 4:
                        pt_group(p_h, g + 2)
                    for w in range(4):
                        t = 4 * g + w
                        nc.tensor.matmul(op0, lhsT=pt_sb[:, t, :],
                                         rhs=vv_all[:, t, 0:512],
                                         start=(t == 0), stop=(t == NT - 1))
                        nc.tensor.matmul(op1, lhsT=pt_sb[:, t, :],
                                         rhs=vv_all[:, t, 512:D],
                                         start=(t == 0), stop=(t == NT - 1))
                nc.vector.tensor_scalar_mul(osb[:, 0:512], op0, linv)
                nc.vector.tensor_scalar_mul(osb[:, 512:D], op1, linv)
                nc.sync.dma_start(y[i * 128:(i + 1) * 128, :], osb)

            sm = [None] * NI

            def E_tile(i, jbs):
                if jbs[0] == 0:
                    st_mx8[i % 2] = smp.tile([128, NJB], FP32, name=f"mx8_{i}",
                                             tag=f"mx8{i % 2}")
                for jb in jbs:
                    if jb % 4 == 0:
                        eq_t[i % 2][jb // 4] = esp.tile(
                            [128, 2048], FP32, name=f"e{i}_{jb // 4}", tag="e")
                    e_block(i, jb)

            # head: E(0)/E(1) interleaved jb-major over the x^T chunk
            # supply; E(0) finishes first so exp(0) starts early
            for jb in range(6):
                E_tile(0, [jb])
                E_tile(1, [jb])
            E_tile(0, [6, 7])
            sm[0] = softmax_issue(0)
            E_tile(1, [6, 7])
            # lag-2 pipeline: E(i); exps(i-1) (after E(i)'s psum copies so
            # the ACT queue drains in dependency order); sweep(i-2)
            for i in range(2, NI):
                E_tile(i, list(range(NJB)))
                if i + 2 < NI:
                    gv_t[i + 2] = qtp.tile([128, NK, 128], FP16,
                                           name=f"gv{i + 2}", tag="gv")
                    nc.sync.dma_start(gv_t[i + 2], gt2[i + 2])
                sm[i - 1] = softmax_issue(i - 1)
                pv_sweep(i - 2, sm[i - 2][0], sm[i - 2][1])
            sm[NI - 1] = softmax_issue(NI - 1)
            pv_sweep(NI - 2, sm[NI - 2][0], sm[NI - 2][1])
            pv_sweep(NI - 1, sm[NI - 1][0], sm[NI - 1][1], last=True)
    nc.compile()
    return nc


def _get_programs():
    if "nc1" not in _cache:
        _cache["nc1"] = _build_phase1()
        _cache["nc2"] = _build_phase2()
    return _cache["nc1"], _cache["nc2"]


def kernel(x, Wq, Wk, Wv, Wo):
    from concourse.bass_utils import run_bass_kernel_spmd

    nc1, nc2 = _get_programs()

    x = np.asarray(x, dtype=np.float32)
    # fold the weights once on the host (associativity):
    #   energy = x (Wq Wk^T) x^T ;  out = attn (x (Wv Wo))
    wm = (np.asarray(Wq, np.float64) @ np.asarray(Wk, np.float64).T
          ).astype(np.float32)
    wvo = (np.asarray(Wv, np.float64) @ np.asarray(Wo, np.float64)
           ).astype(np.float32)

    # ---- phase 1: per-core row slices ----
    in1 = []
    for c in range(8):
        b, i = divmod(c, 4)
        rows = x[b, i * BLK:(i + 1) * BLK, :]           # [BLK, D]
        in1.append({
            "xt": np.ascontiguousarray(rows.T),
            "wm": wm, "wvo": wvo,
        })
    res1 = run_bass_kernel_spmd(nc1, in1, list(range(8))).results

    # ---- host gather of V' shards; pre-cast x^T per batch ----
    xth_full, v_full = [], []
    for b in range(B):
        xth_full.append(np.ascontiguousarray(
            x[b].T.astype(np.float16)))                  # [D, S]
        v = np.concatenate(
            [res1[b * 4 + i]["vo"] for i in range(4)], axis=0)    # [NT, 128, D]
        v_full.append(np.ascontiguousarray(v.transpose(1, 0, 2)))  # [128, NT, D]

    # ---- phase 2 ----
    in2 = []
    for c in range(8):
        b, i = divmod(c, 4)
        gstack = res1[c]["gt"].reshape(NK, 128, NI, 128)  # [n, p, i, f]
        in2.append({
            "xth": xth_full[b], "vin": v_full[b],
            "gt2": np.ascontiguousarray(gstack.transpose(2, 1, 0, 3)),
        })
    res2 = run_bass_kernel_spmd(nc2, in2, list(range(8))).results

    out = np.empty((B, S, D), dtype=np.float32)
    for c in range(8):
        b, i = divmod(c, 4)
        out[b, i * BLK:(i + 1) * BLK, :] = res2[c]["y"].astype(np.float32)
    return out


# revision 43
# speedup vs baseline: 2.2555x; 1.0129x over previous
"""Trainium2 Bass kernel for single-head self-attention (B=2, S=4096, D=1024).

reference:
    q = x @ Wq; k = x @ Wk; v = x @ Wv          # [B,S,D]
    energy = einsum('bid,bjd->bij', q, k) * 8.0  # SCALE = sqrt(64)
    attn = softmax(energy, axis=-1)
    out = einsum('bij,bjd->bid', attn, v) @ Wo

Weight folding (associativity): energy = x @ (Wq Wk^T) @ x^T and
out = attn @ (x @ (Wv Wo)), so the host precomputes M = Wq @ Wk^T and
W' = Wv @ Wo once (fp64) and the device only runs TWO projections
(G = x@M, V' = x@W') plus the two S^2-sized attention matmuls -- the
separate K projection and the output projection disappear.

Two SPMD launches over 8 cores (= 2 batches x 4 query-blocks of 1024):
  phase 1: each core computes G / V' for its own 1024 rows; the host
           gathers V' shards (and pre-casts x^T to fp16) per batch.
  phase 2: each core computes softmax(G_blk @ x^T * 8) @ V' for its
           1024 queries against the full batch; output rows come out
           of the P@V' accumulation directly.

Precision: logits have std ~256 (SCALE multiplies), so the logit path
needs much better than bf16 input precision.  The G projection runs in
the PE's fp32r mode (single-pass at the bf16 rate for free-dim >= 256,
~13 effective mantissa bits); G and x^T are stored as fp16 (2^-11
rounding) and G @ x^T runs in fp16 (fp16 products are exact in fp32
PSUM accumulation).  Combined logit noise ~0.11 std -> ~6e-3 output
rel err.  V' / P run in bf16.

Phase-2 layout: x^T (fp16, 64KB/part) and V' (bf16, 64KB/part) are
SBUF-resident, so after the initial load the attention sweep runs with
no input DMA.  A dma_start occupies its issuing queue through the
whole transfer, so queue assignment is part of the schedule: bulk
loads ride SP in consumption order, the ACT queue stays clear for the
softmax exp chain, and E psum->SBUF copies + P^T copies run on Pool.
"""

import numpy as np
import ml_dtypes

B, S, D = 2, 4096, 1024
BLK = 1024          # queries per core
SCALE = 8.0         # HEAD_DIM ** 0.5 = sqrt(64)
NK = D // 128       # 8 k-tiles over the feature dim
NT = S // 128       # 32 j-tiles over keys
NI = BLK // 128     # 8 i-tiles over this core's queries
NJB = S // 512      # 8 key blocks of 512
BF16 = ml_dtypes.bfloat16

_cache = {}


def _build_phase1():
    """G = x@M and V' = x@W' for this core's 1024 rows (fp32r single-pass)."""
    import concourse.mybir as mybir
    from concourse import bacc
    from concourse.tile import TileContext

    FP16 = mybir.dt.float16
    FP32 = mybir.dt.float32
    FP32R = mybir.dt.float32r
    DBF = mybir.dt.bfloat16

    nc = bacc.Bacc("TRN2", target_bir_lowering=False, debug=False, num_devices=8)

    xt = nc.dram_tensor("xt", [D, BLK], FP16, kind="ExternalInput")  # rows.T
    wm = nc.dram_tensor("wm", [D, D], FP16, kind="ExternalInput")    # Wq@Wk^T
    wvo = nc.dram_tensor("wvo", [D, D], FP16, kind="ExternalInput")  # Wv@Wo
    gt = nc.dram_tensor("gt", [D, BLK], FP16, kind="ExternalOutput")
    vo = nc.dram_tensor("vo", [NI, 128, D], FP16, kind="ExternalOutput")

    with TileContext(nc) as tc:
      with (
          tc.tile_pool(name="xp", bufs=1) as xp,
          tc.tile_pool(name="wp", bufs=1) as wp,
          tc.tile_pool(name="gps", bufs=4, space="PSUM") as gps,
          tc.tile_pool(name="gst", bufs=1) as gstp,
          tc.tile_pool(name="vps", bufs=2, space="PSUM") as vps,
          tc.tile_pool(name="vsb", bufs=3) as vsbp,
      ):
        # all loads on the SP queue in consumption order: the shared DMA
        # bus serializes transfers, so a single queue in priority order
        # beats spreading (racing queues invert priorities)
        xt_r = xt[:, :].rearrange("(n p) s -> p n s", p=128)
        x_sb = xp.tile([128, NK, BLK], FP16, name="x_sb", tag="x_sb")
        wm_sb = wp.tile([128, NK, D], FP16, name="wm_sb", tag="wm_sb")
        wm_r = wm[:, :].rearrange("(n p) d -> p n d", p=128)
        nc.sync.dma_start(x_sb[:, :, 0:256], xt_r[:, :, 0:256])
        nc.sync.dma_start(wm_sb[:, :, 0:256], wm_r[:, :, 0:256])
        nc.sync.dma_start(x_sb[:, :, 256:512], xt_r[:, :, 256:512])
        nc.sync.dma_start(wm_sb[:, :, 256:512], wm_r[:, :, 256:512])
        nc.sync.dma_start(x_sb[:, :, 512:BLK], xt_r[:, :, 512:BLK])
        nc.sync.dma_start(wm_sb[:, :, 512:768], wm_r[:, :, 512:768])
        nc.sync.dma_start(wm_sb[:, :, 768:D], wm_r[:, :, 768:D])
        wv_sb = wp.tile([128, NK, D], FP16, name="wv_sb", tag="wv_sb")
        wvo_r = wvo[:, :].rearrange("(n p) d -> p n d", p=128)
        nc.sync.dma_start(wv_sb[:, :, 0:512], wvo_r[:, :, 0:512])
        nc.sync.dma_start(wv_sb[:, :, 512:D], wvo_r[:, :, 512:D])

        # G blocks emitted in DMA-supply order: each group becomes runnable
        # as one more of the loads above lands.
        K_ORDER = [
            (0, 0), (0, 1),
            (1, 0), (1, 1),
            (0, 2), (0, 3), (1, 2), (1, 3),
            (2, 0), (2, 1), (2, 2), (2, 3),
            (0, 4), (0, 5), (1, 4), (1, 5), (2, 4), (2, 5),
            (0, 6), (0, 7), (1, 6), (1, 7), (2, 6), (2, 7),
        ]
        NBS = ((0, 256), (256, 256), (512, 512))
        st = []
        for m in range(NK):
            st.append(gstp.tile([128, BLK], FP16, name=f"gs{m}", tag=f"s{m}"))
        for (nb, m) in K_ORDER:
            n0, nw = NBS[nb]
            nsl = slice(n0, n0 + nw)
            msl = slice(m * 128, (m + 1) * 128)
            ps = gps.tile([128, 512], FP32, name=f"gp{n0}_{m}", tag="ps")
            for k in range(NK):
                nc.tensor.matmul(ps[:, 0:nw], lhsT=wm_sb[:, k, msl],
                                 rhs=x_sb[:, k, nsl],
                                 start=(k == 0), stop=(k == NK - 1))
            nc.vector.tensor_copy(st[m][:, nsl], ps[:, 0:nw])
        for m in range(NK):
            nc.sync.dma_start(gt[m * 128:(m + 1) * 128, :], st[m])

        for j in range(NI):
            jsl = slice(j * 128, (j + 1) * 128)
            vt = vsbp.tile([128, D], FP16, name=f"vt{j}", tag="vt")
            for db in range(2):
                ps = vps.tile([128, 512], FP32, name=f"vps{j}_{db}", tag="vps")
                for k in range(NK):
                    nc.tensor.matmul(
                        ps, lhsT=x_sb[:, k, jsl],
                        rhs=wv_sb[:, k, db * 512:(db + 1) * 512],
                        start=(k == 0), stop=(k == NK - 1),
                    )
                nc.vector.tensor_copy(vt[:, db * 512:(db + 1) * 512], ps)
                nc.scalar.dma_start(vo[j][:, db * 512:(db + 1) * 512],
                                    vt[:, db * 512:(db + 1) * 512])
    nc.compile()
    return nc


def _build_phase2():
    """softmax(G_blk @ x^T * 8) @ V' for this core's 1024 queries."""
    import concourse.mybir as mybir
    from concourse import bacc
    from concourse.tile import TileContext
    from concourse.masks import make_identity

    FP16 = mybir.dt.float16
    FP32 = mybir.dt.float32
    DBF = mybir.dt.bfloat16
    Exp = mybir.ActivationFunctionType.Exp
    Copy = mybir.ActivationFunctionType.Copy
    AX = mybir.AxisListType.X

    nc = bacc.Bacc("TRN2", target_bir_lowering=False, debug=False, num_devices=8)

    xth = nc.dram_tensor("xth", [D, S], FP16, kind="ExternalInput")
    # per-i-tile partition-major G: [i, p, n, f] = gt[n*128+p, i*128+f]
    gt2 = nc.dram_tensor("gt2", [NI, 128, NK, 128], FP16, kind="ExternalInput")
    # partition-major V': [p, t, d] = V'[t*128+p, d]
    vin = nc.dram_tensor("vin", [128, NT, D], FP16, kind="ExternalInput")
    y = nc.dram_tensor("y", [BLK, D], FP16, kind="ExternalOutput")

    from contextlib import ExitStack
    with TileContext(nc) as tc:
        with ExitStack() as stack:
            constp = stack.enter_context(tc.tile_pool(name="const", bufs=1))
            ident = constp.tile([128, 128], FP16)
            make_identity(nc, ident)

            ktp = stack.enter_context(tc.tile_pool(name="ktp", bufs=1))
            qtp = stack.enter_context(tc.tile_pool(name="qtp", bufs=4))
            vvp = stack.enter_context(tc.tile_pool(name="vvp", bufs=1))
            epsp = stack.enter_context(tc.tile_pool(name="eps", bufs=2, space="PSUM"))
            tpsp = stack.enter_context(tc.tile_pool(name="tps", bufs=2, space="PSUM"))
            opsp = stack.enter_context(tc.tile_pool(name="ops", bufs=2, space="PSUM"))
            smp = stack.enter_context(tc.tile_pool(name="smp", bufs=2))
            esp = stack.enter_context(tc.tile_pool(name="esp", bufs=5))
            pp = stack.enter_context(tc.tile_pool(name="pp", bufs=2))
            ptp = stack.enter_context(tc.tile_pool(name="ptp", bufs=1))
            obp = stack.enter_context(tc.tile_pool(name="obp", bufs=2))

            gv_t = [None] * NI

            def gv(i):
                return gv_t[i]

            gv_t[0] = qtp.tile([128, NK, 128], FP16, name="gv0", tag="gv")
            nc.sync.dma_start(gv_t[0], gt2[0])
            # x^T as one [128, NK, S] tile: each column chunk is a single
            # batched DMA covering all 8 k-rows (full bus bandwidth, one
            # queue slot); everything rides SP in consumption order
            xth_r = xth[:, :].rearrange("(n p) s -> p n s", p=128)
            xth_all = ktp.tile([128, NK, S], FP16, name="xth_all", tag="xth")
            xth_sb = [xth_all[:, m, :] for m in range(NK)]
            vv_all = vvp.tile([128, NT, D], FP16, name="vv_all", tag="vv")
            nc.sync.dma_start(xth_all[:, :, 0:512], xth_r[:, :, 0:512])
            for i in range(1, 4):
                gv_t[i] = qtp.tile([128, NK, 128], FP16, name=f"gv{i}",
                                   tag="gv")
                nc.sync.dma_start(gv_t[i], gt2[i])
            nc.sync.dma_start(xth_all[:, :, 512:1536], xth_r[:, :, 512:1536])
            nc.sync.dma_start(xth_all[:, :, 1536:3072], xth_r[:, :, 1536:3072])
            nc.sync.dma_start(xth_all[:, :, 3072:4096], xth_r[:, :, 3072:4096])
            nc.sync.dma_start(vv_all[:, 0:16, :], vin[:, 0:16, :])
            nc.sync.dma_start(vv_all[:, 16:NT, :], vin[:, 16:NT, :])

            st_mx8 = [None, None]
            eq_t = [[None, None], [None, None]]   # [i%2][half]

            def e_block(i, jb):
                sl = slice(jb * 512, (jb + 1) * 512)
                ps = epsp.tile([128, 512], FP32, name=f"eps{i}_{jb}", tag="eps")
                for k in range(NK):
                    nc.tensor.matmul(ps, lhsT=gv(i)[:, k, :],
                                     rhs=xth_sb[k][:, sl],
                                     start=(k == 0), stop=(k == NK - 1))
                half = jb % 4
                nc.scalar.activation(
                    eq_t[i % 2][jb // 4][:, half * 512:(half + 1) * 512], ps,
                    Copy)
                nc.vector.reduce_max(st_mx8[i % 2][:, jb:jb + 1], ps, axis=AX)

            def softmax_issue(i):
                """Global max + exp chain (DVE stats + ACT exps) for i."""
                mx8 = st_mx8[i % 2]
                mrow = smp.tile([128, 1], FP32, name=f"mrow{i}", tag="mrow")
                nc.vector.reduce_max(mrow, mx8, axis=AX)
                negm = smp.tile([128, 1], FP32, name=f"negm{i}", tag="negm")
                nc.vector.tensor_scalar_mul(negm, mrow, -SCALE)
                # P in two half-row tiles so the next i's exp can start as
                # soon as the first half's transposes have consumed it
                p_h = [pp.tile([128, S // 2], FP16, name=f"p{i}_{h}", tag="p")
                       for h in range(2)]
                lp8 = smp.tile([128, NJB], FP32, name=f"lp8_{i}", tag="lp8")
                for jb in range(NJB):
                    half = jb % 4
                    nc.scalar.activation(
                        p_h[jb // 4][:, (jb % 4) * 512:(jb % 4) * 512 + 512],
                        eq_t[i % 2][jb // 4][:, half * 512:(half + 1) * 512],
                        Exp, bias=negm, scale=SCALE,
                        accum_out=lp8[:, jb:jb + 1],
                    )
                lrow = smp.tile([128, 1], FP32, name=f"lrow{i}", tag="lrow")
                nc.vector.reduce_sum(lrow, lp8, axis=AX)
                linv = smp.tile([128, 1], FP32, name=f"linv{i}", tag="linv")
                nc.vector.reciprocal(linv, lrow)
                return p_h, linv

            _uid = [0]
            pt_sb = ptp.tile([128, NT, 128], FP16, name="pt", tag="pt")

            def pt_group(p_h, g):
                """Transpose 4 P tiles (4g..4g+3) via one psum bank group."""
                _uid[0] += 1
                tp = tpsp.tile([128, 512], FP16, name=f"tpg{_uid[0]}", tag="tp")
                for w in range(4):
                    tl = (4 * g + w) % 16
                    nc.tensor.transpose(tp[:, w * 128:(w + 1) * 128],
                                        p_h[g // 4][:, tl * 128:(tl + 1) * 128],
                                        ident)
                nc.vector.tensor_copy(
                    pt_sb[:, 4 * g:4 * g + 4, :].rearrange("p t f -> p (t f)"), tp)

            def pv_sweep(i, p_h, linv, last=False):
                """P^T + P@V' + 1/l scale + row store for i-tile i.

                P^T groups are issued two ahead of their consuming matmuls
                so the psum->pt copy latency hides under PE work.  For the
                last i-tile the two output halves run as separate t-sweeps
                so half 0 stores while half 1 computes (shorter drain).
                """
                op0 = opsp.tile([128, 512], FP32, name=f"op0_{i}", tag="op0")
                op1 = opsp.tile([128, 512], FP32, name=f"op1_{i}", tag="op1")
                osb = obp.tile([128, D], FP16, name=f"osb{i}", tag="osb")
                halves = ((op0, 0, 512), (op1, 512, D)) if last else None
                if last:
                    pt_group(p_h, 0)
                    pt_group(p_h, 1)
                    for (op, d0, d1) in halves:
                        for t in range(NT):
                            g = t // 4
                            if t % 4 == 0 and g + 2 < NT // 4 and op is op0:
                                pt_group(p_h, g + 2)
                            nc.tensor.matmul(op, lhsT=pt_sb[:, t, :],
                                             rhs=vv_all[:, t, d0:d1],
                                             start=(t == 0), stop=(t == NT - 1))
                        nc.vector.tensor_scalar_mul(osb[:, d0:d1], op, linv)
                        nc.sync.dma_start(y[i * 128:(i + 1) * 128, d0:d1],
                                          osb[:, d0:d1])
                    return
                pt_group(p_h, 0)
                pt_group(p_h, 1)
                for g in range(NT // 4):
                    if g + 2 < NT // 4:
                        pt_group(p_h, g + 2)
                    for w in range(4):
                        t = 4 * g + w
                        nc.tensor.matmul(op0, lhsT=pt_sb[:, t, :],
                                         rhs=vv_all[:, t, 0:512],
                                         start=(t == 0), stop=(t == NT - 1))
                        nc.tensor.matmul(op1, lhsT=pt_sb[:, t, :],
                                         rhs=vv_all[:, t, 512:D],
                                         start=(t == 0), stop=(t == NT - 1))
                nc.vector.tensor_scalar_mul(osb[:, 0:512], op0, linv)
                nc.vector.tensor_scalar_mul(osb[:, 512:D], op1, linv)
                nc.sync.dma_start(y[i * 128:(i + 1) * 128, :], osb)

            sm = [None] * NI

            def E_tile(i, jbs):
                if jbs[0] == 0:
                    st_mx8[i % 2] = smp.tile([128, NJB], FP32, name=f"mx8_{i}",
                                             tag=f"mx8{i % 2}")
                for jb in jbs:
                    if jb % 4 == 0:
                        eq_t[i % 2][jb // 4] = esp.tile(
                            [128, 2048], FP32, name=f"e{i}_{jb // 4}", tag="e")
                    e_block(i, jb)

            # head: E(0)/E(1) interleaved jb-major over the x^T chunk
            # supply; E(0) finishes first so exp(0) starts early
            for jb in range(6):
                E_tile(0, [jb])
                E_tile(1, [jb])
            E_tile(0, [6, 7])
            sm[0] = softmax_issue(0)
            E_tile(1, [6, 7])
            # lag-2 pipeline: E(i); exps(i-1) (after E(i)'s psum copies so
            # the ACT queue drains in dependency order); sweep(i-2)
            for i in range(2, NI):
                E_tile(i, list(range(NJB)))
                if i + 2 < NI:
                    gv_t[i + 2] = qtp.tile([128, NK, 128], FP16,
                                           name=f"gv{i + 2}", tag="gv")
                    nc.sync.dma_start(gv_t[i + 2], gt2[i + 2])
                sm[i - 1] = softmax_issue(i - 1)
                pv_sweep(i - 2, sm[i - 2][0], sm[i - 2][1])
            sm[NI - 1] = softmax_issue(NI - 1)
            pv_sweep(NI - 2, sm[NI - 2][0], sm[NI - 2][1])
            pv_sweep(NI - 1, sm[NI - 1][0], sm[NI - 1][1], last=True)
    nc.compile()
    return nc


def _get_programs():
    if "nc1" not in _cache:
        _cache["nc1"] = _build_phase1()
        _cache["nc2"] = _build_phase2()
    return _cache["nc1"], _cache["nc2"]


def kernel(x, Wq, Wk, Wv, Wo):
    from concourse.bass_utils import run_bass_kernel_spmd

    nc1, nc2 = _get_programs()

    x = np.asarray(x, dtype=np.float32)
    # fold the weights once on the host (associativity):
    #   energy = x (Wq Wk^T) x^T ;  out = attn (x (Wv Wo))
    wm = (np.asarray(Wq, np.float64) @ np.asarray(Wk, np.float64).T
          ).astype(np.float16)
    wvo = (np.asarray(Wv, np.float64) @ np.asarray(Wo, np.float64)
           ).astype(np.float16)

    # ---- phase 1: per-core row slices ----
    in1 = []
    for c in range(8):
        b, i = divmod(c, 4)
        rows = x[b, i * BLK:(i + 1) * BLK, :]           # [BLK, D]
        in1.append({
            "xt": np.ascontiguousarray(rows.T.astype(np.float16)),
            "wm": wm, "wvo": wvo,
        })
    res1 = run_bass_kernel_spmd(nc1, in1, list(range(8))).results

    # ---- host gather of V' shards; pre-cast x^T per batch ----
    xth_full, v_full = [], []
    for b in range(B):
        xth_full.append(np.ascontiguousarray(
            x[b].T.astype(np.float16)))                  # [D, S]
        v = np.concatenate(
            [res1[b * 4 + i]["vo"] for i in range(4)], axis=0)    # [NT, 128, D]
        v_full.append(np.ascontiguousarray(v.transpose(1, 0, 2)))  # [128, NT, D]

    # ---- phase 2 ----
    in2 = []
    for c in range(8):
        b, i = divmod(c, 4)
        gstack = res1[c]["gt"].reshape(NK, 128, NI, 128)  # [n, p, i, f]
        in2.append({
            "xth": xth_full[b], "vin": v_full[b],
            "gt2": np.ascontiguousarray(gstack.transpose(2, 1, 0, 3)),
        })
    res2 = run_bass_kernel_spmd(nc2, in2, list(range(8))).results

    out = np.empty((B, S, D), dtype=np.float32)
    for c in range(8):
        b, i = divmod(c, 4)
        out[b, i * BLK:(i + 1) * BLK, :] = res2[c]["y"].astype(np.float32)
    return out
